# revision 8
# baseline (speedup 1.0000x reference)
"""Trainium2 Bass kernel for BasicEuclideanDistModel (gnn_message_passing).

Math:
  result = sum_e (beta - ||dz_e + dv_e t_e||)
           - dt * sum_{i<j, s} exp(beta - ||z_i(t_s) - z_j(t_s)||)

Device strategy (8 cores, data parallel):
  * Non-event term: full NxN pairwise distances (halved on host).
    d^2(i,j,s) = r_i(s) + r_j(s) - 2 x_i(s)x_j(s) - 2 y_i(s)y_j(s)
    decomposes into a K=8 inner product  F_i(s) . G_j  where G_j is
    time-INdependent:
      F_i(s) = [r_i(s), 1, t_s, t_s^2, -2x_i(s), -2t_s x_i(s), -2y_i(s), -2t_s y_i(s)]
      G_j    = [1,  a_j, b_j, c_j,  zx_j, vx_j, zy_j, vy_j]
    with r(s) = a + b t + c t^2, a = zx^2+zy^2, b = 2(zx vx + zy vy),
    c = vx^2+vy^2.  One [8,128]x[8,2048] matmul (fp32r) per (i-tile, s)
    computes the d^2 supertile; DVE relu clamps rounding negatives,
    ACT computes sqrt then exp(-d) with fused per-partition row sums.
    Each core owns 2 of the 16 i-tiles (rows), all j, all 10 samples.
  * Event term: 25000 events/core, grouped by u-node into 8-slot
    segments.  Both sides are fetched with gpsimd indirect DMA (16B
    descriptors, one 4-float row per index): one descriptor per
    SEGMENT for the u side, one per real event for the v side.  Pad
    slots carry an out-of-bounds index and are skipped by the DMA
    bounds check (their output stays memset-0); a host 0/1 mask
    zeroes their d^2 before the sqrt row-sum.
  * beta enters only as a scalar factor / offset -> folded in on host:
    sum exp(beta - d) = e^beta * sum exp(-d);  sum(beta - d) = E*beta - sum d.
  Host combines 8 cores' [128, 24] partial-sum tensors (pure unshard/
  reduction of partials).
"""

import os
import numpy as np


def _import_concourse():
    try:
        import concourse  # noqa: F401
    except ImportError:
        import sys

        for p in ("/opt/trn_rl_repo", "/root/.axon_site/_ro/trn_rl_repo"):
            if os.path.isdir(p) and p not in sys.path:
                sys.path.insert(0, p)


_import_concourse()

from contextlib import ExitStack  # noqa: E402

import concourse.bacc as bacc  # noqa: E402
import concourse.bass as bass  # noqa: E402
import concourse.mybir as mybir  # noqa: E402
import concourse.tile as tile  # noqa: E402
from concourse.tile_rust import add_dep_helper  # noqa: E402

N = 2048          # nodes
S = 10            # Riemann samples
NCORES = 8
ITILES = 2        # 128-row i-tiles per core
EV_PER_CORE = 200000 // NCORES       # real events per core
# Event layout: events grouped by u-node into segments of SLOT slots; the
# u-side row is fetched once per SEGMENT and broadcast across its slots.
SLOT = 8          # event slots per segment
SP = 36           # segments per partition
C_EV = SP * SLOT                     # 288 event columns per partition
EV_CHUNKS = 4     # v-side indirect-DMA ops per core
EV_CC = C_EV // EV_CHUNKS            # 72 event columns per chunk
SEG_CC = SP // EV_CHUNKS             # 9 segments per chunk
PAD_IDX = 0       # pad slots fetch node 0's row; the mask zeroes their d^2

F32 = mybir.dt.float32
F32R = mybir.dt.float32r
BF16 = mybir.dt.bfloat16
I32 = mybir.dt.int32
AF = mybir.ActivationFunctionType
OP = mybir.AluOpType

_CACHE: dict = {}
DEBUG_DUMP = False  # dev only: dump gathered tiles to DRAM outputs


def _tt(nc, out, in0, in1, op):
    return nc.vector.tensor_tensor(out, in0, in1, op=op)


def _build():
    if "nc" in _CACHE:
        return _CACHE["nc"]

    nc = bacc.Bacc(
        "TRN2", target_bir_lowering=False, debug=False, enable_asserts=False,
    )

    zv_all = nc.dram_tensor("zv_all", [N, 4], F32, kind="ExternalInput").ap()
    zv_i = nc.dram_tensor("zv_i", [ITILES * 128, 4], F32, kind="ExternalInput").ap()
    seg_off_d = nc.dram_tensor("seg_off", [128, SP], I32, kind="ExternalInput").ap()
    ev_off_d = nc.dram_tensor("ev_off", [128, C_EV], I32, kind="ExternalInput").ap()
    ev_t = nc.dram_tensor("ev_t", [128, C_EV], F32, kind="ExternalInput").ap()
    ev_m = nc.dram_tensor("ev_m", [128, C_EV], F32, kind="ExternalInput").ap()
    tb_d = nc.dram_tensor("tb", [128, S], F32, kind="ExternalInput").ap()
    t2b_d = nc.dram_tensor("t2b", [128, S], F32, kind="ExternalInput").ap()
    ident_d = nc.dram_tensor("ident", [128, 128], F32, kind="ExternalInput").ap()
    out_p = nc.dram_tensor("out_p", [128, 24], F32, kind="ExternalOutput").ap()
    if DEBUG_DUMP:
        dbg_seg = nc.dram_tensor("dbg_seg", [128, SP, 4], F32,
                                 kind="ExternalOutput").ap()
        dbg_b = nc.dram_tensor("dbg_b", [128, C_EV, 4], F32,
                               kind="ExternalOutput").ap()
        dbg_d2 = nc.dram_tensor("dbg_d2", [128, C_EV], F32,
                                kind="ExternalOutput").ap()

    with tile.TileContext(nc) as tc, ExitStack() as ctx:
        cpool = ctx.enter_context(tc.tile_pool(name="const", bufs=1))
        evpool = ctx.enter_context(tc.tile_pool(name="ev", bufs=1))

        # ---------------- input loads ----------------
        # event index loads first so the indirect gathers can start ASAP
        soff_sb = evpool.tile([128, SP], I32)
        nc.sync.dma_start(soff_sb[:], seg_off_d)
        voff_sb = evpool.tile([128, C_EV], I32)
        nc.sync.dma_start(voff_sb[:], ev_off_d)
        t_sb = evpool.tile([128, C_EV], F32)
        nc.sync.dma_start(t_sb[:], ev_t)
        m_sb = evpool.tile([128, C_EV], F32)
        nc.sync.dma_start(m_sb[:], ev_m)
        zv_sb = cpool.tile([128, 16, 4], F32)        # all nodes, j-side
        nc.sync.dma_start(zv_sb[:], zv_all.rearrange("(c p) d -> p c d", p=128))
        zvi_sb = cpool.tile([128, ITILES, 4], F32)   # this core's i rows
        nc.sync.dma_start(zvi_sb[:], zv_i.rearrange("(c p) d -> p c d", p=128))
        tb = cpool.tile([128, S], F32)
        nc.sync.dma_start(tb[:], tb_d)
        t2b = cpool.tile([128, S], F32)
        nc.sync.dma_start(t2b[:], t2b_d)
        ident = cpool.tile([128, 128], F32)
        nc.sync.dma_start(ident[:], ident_d)

        acc = cpool.tile([128, 24], F32)
        nc.vector.memset(acc[:], 0.0)

        # ---------------- event gathers ----------------
        # u-side: one 16B row per SEGMENT; v-side: one 16B row per real
        # event slot.  Pad/unused slots have PAD_IDX > N-1: the bounds
        # check skips their descriptors, leaving the memset-0 output.
        seg = evpool.tile([128, SP, 4], F32)
        nc.vector.memset(seg[:], 0.0)
        b_tiles = []
        for ch in range(EV_CHUNKS):
            B = evpool.tile([128, SEG_CC, SLOT, 4], F32)
            nc.vector.memset(B[:], 0.0)
            b_tiles.append(B)

        nc.gpsimd.indirect_dma_start(
            out=seg[:], out_offset=None, in_=zv_all,
            in_offset=bass.IndirectOffsetOnAxis(ap=soff_sb[:], axis=0),
        )
        for ch in range(EV_CHUNKS):
            nc.gpsimd.indirect_dma_start(
                out=b_tiles[ch][:].rearrange("p q j d -> p (q j) d"),
                out_offset=None, in_=zv_all,
                in_offset=bass.IndirectOffsetOnAxis(
                    ap=voff_sb[:, ch * EV_CC:(ch + 1) * EV_CC], axis=0,
                ),
            )

        d2all = evpool.tile([128, C_EV, 1], F32)

        def emit_event_math(ch, scratch_pool):
            B = b_tiles[ch]
            q0 = ch * SEG_CC
            shape4 = [128, SEG_CC, SLOT, 1]
            tse = (
                t_sb[:, ch * EV_CC:(ch + 1) * EV_CC]
                .rearrange("p (q j) -> p q j", j=SLOT)
                .unsqueeze(3)
            )
            mse = (
                m_sb[:, ch * EV_CC:(ch + 1) * EV_CC]
                .rearrange("p (q j) -> p q j", j=SLOT)
                .unsqueeze(3)
            )

            def sv(d):  # seg channel d view broadcast over the slots
                return (
                    seg[:, q0:q0 + SEG_CC, d:d + 1]
                    .unsqueeze(2)
                    .to_broadcast(shape4)
                )

            def bv(d):  # B channel d view
                return B[:, :, :, d:d + 1]

            # scratch from the main-loop w pool (same tag): the slot-reuse
            # WAR deps place these after the main loop in the DVE stream
            dzx = scratch_pool.tile(shape4, F32, tag="w", name="dzx")
            dvx = scratch_pool.tile(shape4, F32, tag="w", name="dvx")
            dzy = scratch_pool.tile(shape4, F32, tag="w", name="dzy")
            dvy = scratch_pool.tile(shape4, F32, tag="w", name="dvy")
            first = _tt(nc, dzx[:], sv(0), bv(0), OP.subtract)
            _tt(nc, dvx[:], sv(2), bv(2), OP.subtract)
            _tt(nc, dvx[:], dvx[:], tse, OP.mult)
            _tt(nc, dzx[:], dzx[:], dvx[:], OP.add)          # dx
            _tt(nc, dzy[:], sv(1), bv(1), OP.subtract)
            _tt(nc, dvy[:], sv(3), bv(3), OP.subtract)
            _tt(nc, dvy[:], dvy[:], tse, OP.mult)
            _tt(nc, dzy[:], dzy[:], dvy[:], OP.add)          # dy
            _tt(nc, dzx[:], dzx[:], dzx[:], OP.mult)
            _tt(nc, dzy[:], dzy[:], dzy[:], OP.mult)
            _tt(nc, dzx[:], dzx[:], dzy[:], OP.add)          # d^2
            d2v = d2all[:, ch * EV_CC:(ch + 1) * EV_CC, :].rearrange(
                "p (q j) d -> p q j d", j=SLOT
            )
            _tt(nc, d2v, dzx[:], mse, OP.mult)               # mask pads -> 0
            return first

        # ---------------- j features  F[p, chunk, 0:8] ----------------
        # [1, a, b, c, zx, vx, zy, vy]; padded to 32 for the PE transpose
        F = cpool.tile([128, 16, 32], F32)
        zx = zv_sb[:, :, 0:1]
        zy = zv_sb[:, :, 1:2]
        vx = zv_sb[:, :, 2:3]
        vy = zv_sb[:, :, 3:4]
        s1 = cpool.tile([128, 16, 1], F32)
        nc.vector.memset(F[:, :, 0:1], 1.0)
        _tt(nc, F[:, :, 1:2], zx, zx, OP.mult)           # a = zx^2 + zy^2
        _tt(nc, s1[:], zy, zy, OP.mult)
        _tt(nc, F[:, :, 1:2], F[:, :, 1:2], s1[:], OP.add)
        s2 = cpool.tile([128, 16, 1], F32)
        _tt(nc, F[:, :, 2:3], zx, vx, OP.mult)           # b = 2(zx vx + zy vy)
        _tt(nc, s2[:], zy, vy, OP.mult)
        _tt(nc, F[:, :, 2:3], F[:, :, 2:3], s2[:], OP.add)
        nc.vector.tensor_scalar_mul(F[:, :, 2:3], F[:, :, 2:3], 2.0)
        s3 = cpool.tile([128, 16, 1], F32)
        _tt(nc, F[:, :, 3:4], vx, vx, OP.mult)           # c = vx^2 + vy^2
        _tt(nc, s3[:], vy, vy, OP.mult)
        _tt(nc, F[:, :, 3:4], F[:, :, 3:4], s3[:], OP.add)
        nc.vector.tensor_copy(F[:, :, 4:5], zx)
        nc.vector.tensor_copy(F[:, :, 5:6], vx)
        nc.vector.tensor_copy(F[:, :, 6:7], zy)
        nc.vector.tensor_copy(F[:, :, 7:8], vy)

        # ---------------- i features  L[p, it, s, 0:8] ----------------
        # [r, 1, t, t^2, -2x, -2tx, -2y, -2ty]
        L = cpool.tile([128, ITILES, S, 32], F32)
        izx = zvi_sb[:, :, 0:1]
        izy = zvi_sb[:, :, 1:2]
        ivx = zvi_sb[:, :, 2:3]
        ivy = zvi_sb[:, :, 3:4]
        # a, b, c for the i rows: [128, ITILES, 1]
        ia = cpool.tile([128, ITILES, 1], F32)
        ib = cpool.tile([128, ITILES, 1], F32)
        ic = cpool.tile([128, ITILES, 1], F32)
        s4 = cpool.tile([128, ITILES, 1], F32)
        _tt(nc, ia[:], izx, izx, OP.mult)
        _tt(nc, s4[:], izy, izy, OP.mult)
        _tt(nc, ia[:], ia[:], s4[:], OP.add)
        s5 = cpool.tile([128, ITILES, 1], F32)
        _tt(nc, ib[:], izx, ivx, OP.mult)
        _tt(nc, s5[:], izy, ivy, OP.mult)
        _tt(nc, ib[:], ib[:], s5[:], OP.add)
        nc.vector.tensor_scalar_mul(ib[:], ib[:], 2.0)
        s6 = cpool.tile([128, ITILES, 1], F32)
        _tt(nc, ic[:], ivx, ivx, OP.mult)
        _tt(nc, s6[:], ivy, ivy, OP.mult)
        _tt(nc, ic[:], ic[:], s6[:], OP.add)

        def b_i(v):  # [128, ITILES, 1] -> [128, ITILES, S, 1]
            return v.unsqueeze(2).to_broadcast([128, ITILES, S, 1])

        tv = tb.unsqueeze(1).unsqueeze(3).to_broadcast([128, ITILES, S, 1])
        t2v = t2b.unsqueeze(1).unsqueeze(3).to_broadcast([128, ITILES, S, 1])

        nc.vector.memset(L[:, :, :, 1:2], 1.0)
        nc.vector.tensor_copy(L[:, :, :, 2:3], tv)
        nc.vector.tensor_copy(L[:, :, :, 3:4], t2v)
        Lx = cpool.tile([128, ITILES, S, 1], F32)
        _tt(nc, Lx[:], b_i(ivx), tv, OP.mult)            # x_i(s) = zx + vx t
        _tt(nc, Lx[:], Lx[:], b_i(izx), OP.add)
        nc.vector.tensor_scalar_mul(L[:, :, :, 4:5], Lx[:], -2.0)
        _tt(nc, L[:, :, :, 5:6], L[:, :, :, 4:5], tv, OP.mult)
        Ly = cpool.tile([128, ITILES, S, 1], F32)
        _tt(nc, Ly[:], b_i(ivy), tv, OP.mult)
        _tt(nc, Ly[:], Ly[:], b_i(izy), OP.add)
        nc.vector.tensor_scalar_mul(L[:, :, :, 6:7], Ly[:], -2.0)
        _tt(nc, L[:, :, :, 7:8], L[:, :, :, 6:7], tv, OP.mult)
        Lr = cpool.tile([128, ITILES, S, 1], F32)
        _tt(nc, L[:, :, :, 0:1], b_i(ib), tv, OP.mult)   # r = a + b t + c t^2
        _tt(nc, L[:, :, :, 0:1], L[:, :, :, 0:1], b_i(ia), OP.add)
        _tt(nc, Lr[:], b_i(ic), t2v, OP.mult)
        _tt(nc, L[:, :, :, 0:1], L[:, :, :, 0:1], Lr[:], OP.add)

        # ---------------- transposes (PE) ----------------
        # transpose copies write float32r directly (rounds for the fp32r
        # matmul; Bacc's generate_event_semaphores legalizes the waits)
        T2 = cpool.tile([8, N], F32R)                    # G_j rows
        L2 = cpool.tile([8, ITILES * S, 128], F32R)      # F_i(s) rows
        with tc.tile_pool(name="tp", bufs=4, space="PSUM") as tpp:
            for c in range(16):
                pt = tpp.tile([32, 128], F32, tag="pt", name="pt")
                nc.tensor.transpose(pt[:], F[:, c, :], ident[:])
                nc.vector.tensor_copy(T2[:, c * 128:(c + 1) * 128], pt[0:8, :])
            for it in range(ITILES):
                for s in range(S):
                    pt = tpp.tile([32, 128], F32, tag="pt", name="pt")
                    nc.tensor.transpose(pt[:], L[:, it, s, :], ident[:])
                    nc.vector.tensor_copy(L2[:, it * S + s, :], pt[0:8, :])

        d_ev = evpool.tile([128, C_EV, 1], F32)

        # ---------------- main pairwise loop ----------------
        sq_insts = [[] for _ in range(ITILES)]
        ex_insts = [[] for _ in range(ITILES)]
        relu_insts = []
        with tc.tile_pool(name="qp", bufs=2, space="PSUM") as qpool, \
                tc.tile_pool(name="wp", bufs=12) as wpool:
            for it in range(ITILES):
                for s in range(S):
                    q = qpool.tile([128, N], F32, tag="q", name="q")
                    for kk in range(4):
                        nc.tensor.matmul(
                            q[:, kk * 512:(kk + 1) * 512],
                            L2[:, it * S + s, :],
                            T2[:, kk * 512:(kk + 1) * 512],
                            start=True, stop=True,
                        )
                    w = wpool.tile([128, N], BF16, tag="w", name="w")
                    relu_insts.append(
                        nc.vector.tensor_scalar_max(w[:], q[:], 0.0)
                    )
                    col = it * S + s
                    sq = nc.scalar.activation(w[:], w[:], AF.Sqrt)
                    ex = nc.scalar.activation(
                        w[:], w[:], AF.Exp, scale=-1.0,
                        accum_out=acc[:, col:col + 1],
                    )
                    sq_insts[it].append(sq)
                    ex_insts[it].append(ex)

            # event distance algebra AFTER the relus in the DVE stream:
            # its inputs (gathers) complete long after the main loop's
            # DVE work is ready, and engine streams execute in order
            for ch in range(EV_CHUNKS):
                emit_event_math(ch, wpool)

            ev_sq = nc.scalar.activation(
                d_ev[:], d2all[:], AF.Sqrt, accum_out=acc[:, 20:21]
            )

            # Force ACT phase order: sqrt(i0) exp(i0) sqrt(i1) exp(i1) ev.
            # The event gathers land late, so the event sqrt goes last
            # (one extra table load, but no ACT stall).
            order = (
                sq_insts[0] + ex_insts[0] + sq_insts[1] + ex_insts[1] + [ev_sq]
            )
            for a, b in zip(order[1:], order[:-1]):
                add_dep_helper(a.ins, b.ins, reason="act table phase order")

            nc.sync.dma_start(out_p, acc[:])
            if DEBUG_DUMP:
                nc.sync.dma_start(dbg_seg, seg[:])
                for ch in range(EV_CHUNKS):
                    nc.sync.dma_start(
                        dbg_b[:, ch * EV_CC:(ch + 1) * EV_CC, :],
                        b_tiles[ch][:].rearrange("p q j d -> p (q j) d"),
                    )
                nc.sync.dma_start(dbg_d2, d2all[:, :, 0])

    nc.compile()  # wait legalization (1 sync wait / instruction) + act table loads
    _CACHE["nc"] = nc
    return nc


def _marshal(inputs):
    z0 = np.asarray(inputs["z0"], dtype=np.float32)
    v0 = np.asarray(inputs["v0"], dtype=np.float32)
    uv = np.asarray(inputs["data_uv"], dtype=np.int32)
    tt = np.asarray(inputs["data_t"], dtype=np.float32)
    t0 = np.float32(np.asarray(inputs["t0"]).reshape(-1)[0])
    tn = np.float32(np.asarray(inputs["tn"]).reshape(-1)[0])

    zv = np.ascontiguousarray(np.concatenate([z0, v0], axis=1)).astype(np.float32)
    dt = np.float32((tn - t0) / np.float32(S))
    tmid = (t0 + (np.arange(S, dtype=np.float32) + np.float32(0.5)) * dt).astype(
        np.float32
    )
    tb = np.ascontiguousarray(np.broadcast_to(tmid, (128, S))).astype(np.float32)
    t2b = (tb * tb).astype(np.float32)

    E = uv.shape[0]
    assert E <= NCORES * EV_PER_CORE
    u_all = uv[:, 0].astype(np.int32)
    v_all = uv[:, 1].astype(np.int32)

    def pack_events(u, v, t):
        """Group a core's events by u into segments of <= SLOT slots.
        Pad/unused slots get index PAD_IDX (descriptor skipped) and
        mask 0."""
        order = np.argsort(u, kind="stable")
        us, vs, ts = u[order], v[order], t[order]
        starts = np.flatnonzero(np.r_[True, us[1:] != us[:-1]])
        ends = np.r_[starts[1:], len(us)]
        seg_nodes = np.full((128, SP), PAD_IDX, np.int32)
        v_slots = np.full((128, SP, SLOT), PAD_IDX, np.int32)
        t_slots = np.zeros((128, SP, SLOT), np.float32)
        m_slots = np.zeros((128, SP, SLOT), np.float32)
        counts = np.zeros(128, np.int64)
        i = 0
        for s0, e0 in zip(starts, ends):
            n = us[s0]
            for j in range(s0, e0, SLOT):
                p = i % 128
                q = counts[p]
                counts[p] += 1
                assert q < SP, "segment overflow; raise SP"
                i += 1
                seg_nodes[p, q] = n
                va = vs[j:min(j + SLOT, e0)]
                ta = ts[j:min(j + SLOT, e0)]
                v_slots[p, q, : len(va)] = va
                t_slots[p, q, : len(ta)] = ta
                m_slots[p, q, : len(va)] = 1.0
        return (
            seg_nodes,
            v_slots.reshape(128, C_EV),
            t_slots.reshape(128, C_EV),
            m_slots.reshape(128, C_EV),
        )

    ident_np = np.eye(128, dtype=np.float32)
    in_maps = []
    for k in range(NCORES):
        sl = slice(k * EV_PER_CORE, (k + 1) * EV_PER_CORE)
        seg_nodes, v_slots, t_slots, m_slots = pack_events(
            u_all[sl], v_all[sl], tt[sl]
        )
        in_maps.append(
            {
                "zv_all": zv,
                "zv_i": np.ascontiguousarray(zv[k * 256:(k + 1) * 256]),
                "seg_off": seg_nodes,
                "ev_off": v_slots,
                "ev_t": np.ascontiguousarray(t_slots),
                "ev_m": np.ascontiguousarray(m_slots),
                "tb": tb,
                "t2b": t2b,
                "ident": ident_np,
            }
        )
    return in_maps, (float(t0), float(tn), E)


def _np_event_partial(m, zv):
    """Reference (numpy, f64) per-partition event distance sums for one
    core's marshalled inputs — used by the dev test harnesses."""
    seg_nodes = m["seg_off"].astype(np.int64)          # [128, SP]
    v_slots = m["ev_off"].astype(np.int64)             # [128, C_EV]
    t_slots = m["ev_t"].astype(np.float64)
    mask = m["ev_m"].astype(np.float64)
    un = np.repeat(np.clip(seg_nodes, 0, N - 1), SLOT, axis=1)
    vn = np.clip(v_slots, 0, N - 1)
    a = zv[un]
    b = zv[vn]
    dx = (a[..., 0] - b[..., 0]) + (a[..., 2] - b[..., 2]) * t_slots
    dy = (a[..., 1] - b[..., 1]) + (a[..., 3] - b[..., 3]) * t_slots
    return (np.sqrt(dx * dx + dy * dy) * mask).sum(axis=1)


def _combine(core_outs, beta, t0, tn, E):
    """core_outs: list of [128, 24] float32 partial-sum tensors."""
    exp_sum = 0.0
    ev_sum = 0.0
    for o in core_outs:
        o = np.asarray(o, dtype=np.float64)
        exp_sum += o[:, 0 : ITILES * S].sum()
        ev_sum += o[:, 20].sum()
    b = float(beta)
    dt = (tn - t0) / S
    event_intensity = E * b - ev_sum
    non_event = np.exp(b) * (exp_sum - S * N) / 2.0 * dt
    return np.float32(event_intensity - 1.0 * non_event)


def kernel(**inputs) -> np.ndarray:
    from concourse.bass_utils import run_bass_kernel_spmd

    nc = _build()
    in_maps, (t0, tn, E) = _marshal(inputs)
    res = run_bass_kernel_spmd(nc, in_maps, core_ids=list(range(NCORES)))
    beta = float(np.asarray(inputs["beta"]).reshape(-1)[0])
    out = _combine([r["out_p"] for r in res.results], beta, t0, tn, E)
    return np.asarray(out, dtype=np.float32)


# revision 18
# speedup vs baseline: 1.0582x; 1.0582x over previous
"""Trainium2 Bass kernel for BasicEuclideanDistModel (gnn_message_passing).

Math:
  result = sum_e (beta - ||dz_e + dv_e t_e||)
           - dt * sum_{i<j, s} exp(beta - ||z_i(t_s) - z_j(t_s)||)

Device strategy (8 cores, data parallel):
  * Non-event term: full NxN pairwise distances (halved on host).
    d^2(i,j,s) = r_i(s) + r_j(s) - 2 x_i(s)x_j(s) - 2 y_i(s)y_j(s)
    decomposes into a K=8 inner product  F_i(s) . G_j  where G_j is
    time-INdependent.  One [8,128]x[8,2048] matmul (fp32r) per
    (i-tile, s) computes the d^2 supertile; DVE relu clamps rounding
    negatives, ACT computes sqrt then exp(-d) with fused row sums.
    Each core owns 2 of the 16 i-tiles, all j, all 10 samples.
  * Event term: d^2(u,v,t) = sum_{k<14} A_k(u) B_k(t) C_k(v) -- a
    trilinear decomposition with B_k in {1, t, t^2}.  Events are split
    into 8 groups (one per 16-partition Q7 tile); partition 16g+k
    holds channel k.  A single gpsimd ap_gather (SBUF gather, shared
    index list per group, per-partition table) fetches A_k(u_seg) per
    8-event segment and C_k(v_e) per event from a host-built
    [128, 4096] bf16 channel table.  DVE forms P = A*C*T where the
    host T tensor carries B_k(t_e) and the pad mask; PE reduces the
    14 channels with a block-ones stationary into [72, 512] PSUM
    rows; one ACT sqrt row-sum accumulates sum d_e.
  * beta enters only as a scalar factor / offset -> folded in on host.
  Host combines 8 cores' [128, 24] partial-sum tensors (pure unshard/
  reduction of partials).
"""

import os
import numpy as np


def _import_concourse():
    try:
        import concourse  # noqa: F401
    except ImportError:
        import sys

        for p in ("/opt/trn_rl_repo", "/root/.axon_site/_ro/trn_rl_repo"):
            if os.path.isdir(p) and p not in sys.path:
                sys.path.insert(0, p)


_import_concourse()

from contextlib import ExitStack  # noqa: E402

import concourse.bacc as bacc  # noqa: E402
import concourse.mybir as mybir  # noqa: E402
import concourse.tile as tile  # noqa: E402
from concourse.tile_rust import add_dep_helper  # noqa: E402

N = 2048          # nodes
S = 10            # Riemann samples
NCORES = 8
ITILES = 2        # 128-row i-tiles per core
EV_PER_CORE = 200000 // NCORES       # real events per core
NG = 8            # event groups (one per 16-partition Q7 tile)
SLOT = 8          # event slots per segment
SEG_G = 576       # segments per group
EV_G = SEG_G * SLOT                  # 4608 event slots per group
NIDX = SEG_G + EV_G                  # 5184 gather indices per group

F32 = mybir.dt.float32
F32R = mybir.dt.float32r
BF16 = mybir.dt.bfloat16
I16 = mybir.dt.int16
AF = mybir.ActivationFunctionType
OP = mybir.AluOpType

_CACHE: dict = {}


def _tt(nc, out, in0, in1, op):
    return nc.vector.tensor_tensor(out, in0, in1, op=op)


def _build():
    if "nc" in _CACHE:
        return _CACHE["nc"]

    nc = bacc.Bacc(
        "TRN2", target_bir_lowering=False, debug=False, enable_asserts=False,
    )

    zv_all = nc.dram_tensor("zv_all", [N, 4], F32, kind="ExternalInput").ap()
    zv_i = nc.dram_tensor("zv_i", [ITILES * 128, 4], F32, kind="ExternalInput").ap()
    gtab_d = nc.dram_tensor("gtab", [128, N, 2], BF16, kind="ExternalInput").ap()
    gidx_d = nc.dram_tensor("gidx", [128, NIDX // 16], I16,
                            kind="ExternalInput").ap()
    tmat_d = nc.dram_tensor("tmat", [128, EV_G], BF16, kind="ExternalInput").ap()
    ones16_d = nc.dram_tensor("ones16", [128, 8], F32, kind="ExternalInput").ap()
    tb_d = nc.dram_tensor("tb", [128, S], F32, kind="ExternalInput").ap()
    t2b_d = nc.dram_tensor("t2b", [128, S], F32, kind="ExternalInput").ap()
    ident_d = nc.dram_tensor("ident", [128, 128], F32, kind="ExternalInput").ap()
    out_p = nc.dram_tensor("out_p", [128, 24], F32, kind="ExternalOutput").ap()

    with tile.TileContext(nc) as tc, ExitStack() as ctx:
        cpool = ctx.enter_context(tc.tile_pool(name="const", bufs=1))
        evpool = ctx.enter_context(tc.tile_pool(name="ev", bufs=1))

        # ---------------- input loads ----------------
        # gather table + indices first: the one gpsimd ap_gather (and its
        # library load) should start as early as possible
        gtab = evpool.tile([128, N, 2], BF16)
        nc.sync.dma_start(gtab[:], gtab_d)
        gidx = evpool.tile([128, NIDX // 16], I16)
        nc.sync.dma_start(gidx[:], gidx_d)
        tmat = evpool.tile([128, EV_G], BF16)
        nc.sync.dma_start(tmat[:], tmat_d)
        ones16_f = evpool.tile([128, 8], F32)
        nc.sync.dma_start(ones16_f[:], ones16_d)
        zv_sb = cpool.tile([128, 16, 4], F32)        # all nodes, j-side
        nc.sync.dma_start(zv_sb[:], zv_all.rearrange("(c p) d -> p c d", p=128))
        zvi_sb = cpool.tile([128, ITILES, 4], F32)   # this core's i rows
        nc.sync.dma_start(zvi_sb[:], zv_i.rearrange("(c p) d -> p c d", p=128))
        tb = cpool.tile([128, S], F32)
        nc.sync.dma_start(tb[:], tb_d)
        t2b = cpool.tile([128, S], F32)
        nc.sync.dma_start(t2b[:], t2b_d)
        ident = cpool.tile([128, 128], F32)
        nc.sync.dma_start(ident[:], ident_d)

        acc = cpool.tile([128, 24], F32)
        nc.vector.memset(acc[:], 0.0)

        # ---------------- event gather (one gpsimd instruction) ----------
        # table row n = [A_k(n), C_k(n)] (d=2 bf16 = 4B per index); the
        # segment slots use component 0, the event slots component 1
        gout = evpool.tile([128, NIDX, 2], BF16)
        nc.gpsimd.ap_gather(
            gout[:], gtab[:], gidx[:],
            channels=128, num_elems=N, d=2, num_idxs=NIDX,
        )
        ones16 = evpool.tile([128, 8], BF16)
        nc.vector.tensor_copy(ones16[:], ones16_f[:])

        # ---------------- j features  F[p, chunk, 0:8] ----------------
        # [1, a, b, c, zx, vx, zy, vy]; padded to 32 for the PE transpose
        F = cpool.tile([128, 16, 32], F32)
        zx = zv_sb[:, :, 0:1]
        zy = zv_sb[:, :, 1:2]
        vx = zv_sb[:, :, 2:3]
        vy = zv_sb[:, :, 3:4]
        s1 = cpool.tile([128, 16, 1], F32)
        nc.vector.memset(F[:, :, 0:1], 1.0)
        _tt(nc, F[:, :, 1:2], zx, zx, OP.mult)           # a = zx^2 + zy^2
        _tt(nc, s1[:], zy, zy, OP.mult)
        _tt(nc, F[:, :, 1:2], F[:, :, 1:2], s1[:], OP.add)
        s2 = cpool.tile([128, 16, 1], F32)
        _tt(nc, F[:, :, 2:3], zx, vx, OP.mult)           # b = 2(zx vx + zy vy)
        _tt(nc, s2[:], zy, vy, OP.mult)
        _tt(nc, F[:, :, 2:3], F[:, :, 2:3], s2[:], OP.add)
        nc.vector.tensor_scalar_mul(F[:, :, 2:3], F[:, :, 2:3], 2.0)
        s3 = cpool.tile([128, 16, 1], F32)
        _tt(nc, F[:, :, 3:4], vx, vx, OP.mult)           # c = vx^2 + vy^2
        _tt(nc, s3[:], vy, vy, OP.mult)
        _tt(nc, F[:, :, 3:4], F[:, :, 3:4], s3[:], OP.add)
        nc.vector.tensor_copy(F[:, :, 4:5], zx)
        nc.vector.tensor_copy(F[:, :, 5:6], vx)
        nc.vector.tensor_copy(F[:, :, 6:7], zy)
        nc.vector.tensor_copy(F[:, :, 7:8], vy)

        # ---------------- i features  L[p, it, s, 0:8] ----------------
        # [r, 1, t, t^2, -2x, -2tx, -2y, -2ty]
        L = cpool.tile([128, ITILES, S, 32], F32)
        izx = zvi_sb[:, :, 0:1]
        izy = zvi_sb[:, :, 1:2]
        ivx = zvi_sb[:, :, 2:3]
        ivy = zvi_sb[:, :, 3:4]
        ia = cpool.tile([128, ITILES, 1], F32)
        ib = cpool.tile([128, ITILES, 1], F32)
        ic = cpool.tile([128, ITILES, 1], F32)
        s4 = cpool.tile([128, ITILES, 1], F32)
        _tt(nc, ia[:], izx, izx, OP.mult)
        _tt(nc, s4[:], izy, izy, OP.mult)
        _tt(nc, ia[:], ia[:], s4[:], OP.add)
        s5 = cpool.tile([128, ITILES, 1], F32)
        _tt(nc, ib[:], izx, ivx, OP.mult)
        _tt(nc, s5[:], izy, ivy, OP.mult)
        _tt(nc, ib[:], ib[:], s5[:], OP.add)
        nc.vector.tensor_scalar_mul(ib[:], ib[:], 2.0)
        s6 = cpool.tile([128, ITILES, 1], F32)
        _tt(nc, ic[:], ivx, ivx, OP.mult)
        _tt(nc, s6[:], ivy, ivy, OP.mult)
        _tt(nc, ic[:], ic[:], s6[:], OP.add)

        def b_i(v):  # [128, ITILES, 1] -> [128, ITILES, S, 1]
            return v.unsqueeze(2).to_broadcast([128, ITILES, S, 1])

        tv = tb.unsqueeze(1).unsqueeze(3).to_broadcast([128, ITILES, S, 1])
        t2v = t2b.unsqueeze(1).unsqueeze(3).to_broadcast([128, ITILES, S, 1])

        nc.vector.memset(L[:, :, :, 1:2], 1.0)
        nc.vector.tensor_copy(L[:, :, :, 2:3], tv)
        nc.vector.tensor_copy(L[:, :, :, 3:4], t2v)
        Lx = cpool.tile([128, ITILES, S, 1], F32)
        _tt(nc, Lx[:], b_i(ivx), tv, OP.mult)            # x_i(s) = zx + vx t
        _tt(nc, Lx[:], Lx[:], b_i(izx), OP.add)
        nc.vector.tensor_scalar_mul(L[:, :, :, 4:5], Lx[:], -2.0)
        _tt(nc, L[:, :, :, 5:6], L[:, :, :, 4:5], tv, OP.mult)
        Ly = cpool.tile([128, ITILES, S, 1], F32)
        _tt(nc, Ly[:], b_i(ivy), tv, OP.mult)
        _tt(nc, Ly[:], Ly[:], b_i(izy), OP.add)
        nc.vector.tensor_scalar_mul(L[:, :, :, 6:7], Ly[:], -2.0)
        _tt(nc, L[:, :, :, 7:8], L[:, :, :, 6:7], tv, OP.mult)
        Lr = cpool.tile([128, ITILES, S, 1], F32)
        _tt(nc, L[:, :, :, 0:1], b_i(ib), tv, OP.mult)   # r = a + b t + c t^2
        _tt(nc, L[:, :, :, 0:1], L[:, :, :, 0:1], b_i(ia), OP.add)
        _tt(nc, Lr[:], b_i(ic), t2v, OP.mult)
        _tt(nc, L[:, :, :, 0:1], L[:, :, :, 0:1], Lr[:], OP.add)

        # ---------------- transposes (PE) ----------------
        T2 = cpool.tile([8, N], F32R)                    # G_j rows
        L2 = cpool.tile([8, ITILES * S, 128], F32R)      # F_i(s) rows
        with tc.tile_pool(name="tp", bufs=4, space="PSUM") as tpp:
            for c in range(16):
                pt = tpp.tile([32, 128], F32, tag="pt", name="pt")
                nc.tensor.transpose(pt[:], F[:, c, :], ident[:])
                nc.vector.tensor_copy(T2[:, c * 128:(c + 1) * 128], pt[0:8, :])
            for it in range(ITILES):
                for s in range(S):
                    pt = tpp.tile([32, 128], F32, tag="pt", name="pt")
                    nc.tensor.transpose(pt[:], L[:, it, s, :], ident[:])
                    nc.vector.tensor_copy(L2[:, it * S + s, :], pt[0:8, :])

        # ---------------- event P = A * C * T (DVE, bf16) -------------
        P = evpool.tile([128, SEG_G, SLOT, 1], BF16)
        shape4 = [128, SEG_G, SLOT, 1]
        a_view = gout[:, 0:SEG_G, 0:1].unsqueeze(2).to_broadcast(shape4)
        c_view = gout[:, SEG_G:NIDX, 1:2].rearrange(
            "p (q j) d -> p q j d", j=SLOT
        )
        t_view = tmat.rearrange("p (q j) -> p q j", j=SLOT).unsqueeze(3)

        # ---------------- main pairwise loop ----------------
        sq_insts = [[] for _ in range(ITILES)]
        ex_insts = [[] for _ in range(ITILES)]
        ev_dve = []
        with tc.tile_pool(name="qp", bufs=2, space="PSUM") as qpool, \
                tc.tile_pool(name="wp", bufs=12) as wpool:
            ev_sq = None
            for it in range(ITILES):
                for s in range(S):
                    q = qpool.tile([128, N], F32, tag="q", name="q")
                    for kk in range(4):
                        nc.tensor.matmul(
                            q[:, kk * 512:(kk + 1) * 512],
                            L2[:, it * S + s, :],
                            T2[:, kk * 512:(kk + 1) * 512],
                            start=True, stop=True,
                        )
                    w = wpool.tile([128, N], BF16, tag="w", name="w")
                    nc.vector.tensor_scalar_max(w[:], q[:], 0.0)
                    col = it * S + s
                    sq = nc.scalar.activation(w[:], w[:], AF.Sqrt)
                    ex = nc.scalar.activation(
                        w[:], w[:], AF.Exp, scale=-1.0,
                        accum_out=acc[:, col:col + 1],
                    )
                    sq_insts[it].append(sq)
                    ex_insts[it].append(ex)

                if it == 0:
                    # event tail, emitted mid-main-loop: DVE product, PE
                    # 14-channel block reduce (one PSUM tile reused over 3
                    # rounds, PSUM->SBUF copies between), one ACT sqrt
                    # row-sum.  All inputs are ready well before this.
                    ev_dve.append(_tt(nc, P[:], a_view, c_view, OP.mult))
                    ev_dve.append(_tt(nc, P[:], P[:], t_view, OP.mult))
                    q_ev = qpool.tile([128, N], F32, tag="q", name="q")
                    ev_d2 = evpool.tile([8, EV_G], F32)
                    pm = P[:].rearrange("p q j d -> p (q j d)")
                    for r in range(9):
                        c0 = (r % 4) * 512
                        nc.tensor.matmul(
                            q_ev[0:8, c0:c0 + 512],
                            ones16[:],
                            pm[:, 512 * r:512 * (r + 1)],
                            start=True, stop=True,
                        )
                        nc.vector.tensor_scalar_max(
                            ev_d2[:, 512 * r:512 * (r + 1)],
                            q_ev[0:8, c0:c0 + 512],
                            0.0,
                        )
                    w_ev = evpool.tile([8, EV_G], BF16)
                    ev_sq = nc.scalar.activation(
                        w_ev[:], ev_d2[:], AF.Sqrt,
                        accum_out=acc[0:8, 20:21],
                    )

            # ACT phase order: sqrt(i0) exp(i0) sqrt(i1)+ev exp(i1):
            # the event sqrt shares the i1 Sqrt table load.
            order = (
                sq_insts[0] + ex_insts[0] + sq_insts[1] + [ev_sq] + ex_insts[1]
            )
            for a, b in zip(order[1:], order[:-1]):
                add_dep_helper(a.ins, b.ins, reason="act table phase order")

            nc.sync.dma_start(out_p, acc[:])

    nc.compile()
    _CACHE["nc"] = nc
    return nc


# trilinear channels: (A_k(u), B_k(t) power, C_k(v)); a/b/c as in module doc
# columns of the per-node feature matrix: [1, a, b, c, zx, zy, vx, vy]
_ACH = [1, 0, 4, 5, 2, 0, 4, 6, 5, 7, 3, 0, 6, 7]   # A feature index
_ASC = [1., 1., -2., -2., 1., 1., -2., -2., -2., -2., 1., 1., -2., -2.]
_BPOW = [0, 0, 0, 0, 1, 1, 1, 1, 1, 1, 2, 2, 2, 2]  # power of t
_CCH = [0, 1, 4, 5, 0, 2, 6, 4, 7, 5, 0, 3, 6, 7]   # C feature index


def _node_features(zv):
    zx, zy, vx, vy = zv[:, 0], zv[:, 1], zv[:, 2], zv[:, 3]
    a = zx * zx + zy * zy
    b = 2.0 * (zx * vx + zy * vy)
    c = vx * vx + vy * vy
    one = np.ones_like(a)
    return np.stack([one, a, b, c, zx, zy, vx, vy], axis=1)  # [N, 8]


def _marshal(inputs):
    import ml_dtypes

    z0 = np.asarray(inputs["z0"], dtype=np.float32)
    v0 = np.asarray(inputs["v0"], dtype=np.float32)
    uv = np.asarray(inputs["data_uv"], dtype=np.int32)
    tt = np.asarray(inputs["data_t"], dtype=np.float32)
    t0 = np.float32(np.asarray(inputs["t0"]).reshape(-1)[0])
    tn = np.float32(np.asarray(inputs["tn"]).reshape(-1)[0])

    zv = np.ascontiguousarray(np.concatenate([z0, v0], axis=1)).astype(np.float32)
    dt = np.float32((tn - t0) / np.float32(S))
    tmid = (t0 + (np.arange(S, dtype=np.float32) + np.float32(0.5)) * dt).astype(
        np.float32
    )
    tb = np.ascontiguousarray(np.broadcast_to(tmid, (128, S))).astype(np.float32)
    t2b = (tb * tb).astype(np.float32)

    feats = _node_features(zv.astype(np.float64)).astype(np.float32)  # [N, 8]

    # channel table [128, N, 2] bf16: partition 16g+k, row n = [A_k, C_k]
    gtab = np.zeros((128, N, 2), np.float32)
    for k in range(14):
        for g in range(NG):
            gtab[16 * g + k, :, 0] = _ASC[k] * feats[:, _ACH[k]]
            gtab[16 * g + k, :, 1] = feats[:, _CCH[k]]
    gtab = gtab.astype(ml_dtypes.bfloat16)

    E = uv.shape[0]
    assert E <= NCORES * EV_PER_CORE
    u_all = uv[:, 0].astype(np.int64)
    v_all = uv[:, 1].astype(np.int64)

    def pack_core(u, v, t):
        """Split a core's events into NG groups by u-node (balanced),
        group each group's events into <=SLOT segments, and emit the
        wrapped gather index list + the T (B_k(t) * mask) tensor."""
        counts = np.bincount(u, minlength=N)
        order = np.argsort(-counts, kind="stable")
        g_ev = np.zeros(NG, np.int64)
        node_g = np.zeros(N, np.int64)
        for n in order:
            g = int(np.argmin(g_ev))
            node_g[n] = g
            g_ev[g] += counts[n]
        assert g_ev.max() <= EV_G, f"group overflow {g_ev}"

        by_node_order = np.argsort(u * NG + node_g[u], kind="stable")
        us, vs, ts = u[by_node_order], v[by_node_order], t[by_node_order]
        gs = node_g[us]

        seg_u = np.zeros((NG, SEG_G), np.int64)       # u node per segment
        ev_v = np.zeros((NG, SEG_G, SLOT), np.int64)  # v node per slot
        ev_t = np.zeros((NG, SEG_G, SLOT), np.float32)
        ev_m = np.zeros((NG, SEG_G, SLOT), np.float32)
        seg_cnt = np.zeros(NG, np.int64)
        i = 0
        while i < len(us):
            j = i
            while j < len(us) and us[j] == us[i]:
                j += 1
            g = int(gs[i])
            for s0 in range(i, j, SLOT):
                q = seg_cnt[g]
                assert q < SEG_G, "segment overflow; raise SEG_G"
                seg_cnt[g] += 1
                e0 = min(s0 + SLOT, j)
                seg_u[g, q] = us[i]
                ev_v[g, q, : e0 - s0] = vs[s0:e0]
                ev_t[g, q, : e0 - s0] = ts[s0:e0]
                ev_m[g, q, : e0 - s0] = 1.0
            i = j
        # gather index list per group: segment u-nodes then event v-nodes
        idx_flat = np.concatenate(
            [seg_u, ev_v.reshape(NG, EV_G)], axis=1
        ).astype(np.int16)                            # [NG, NIDX]
        gidx = np.zeros((128, NIDX // 16), np.int16)
        for g in range(NG):
            gidx[16 * g:16 * (g + 1), :] = (
                idx_flat[g].reshape(NIDX // 16, 16).T
            )
        # T tensor [128, EV_G]: partition 16g+k: B_k(t_e) * mask_e
        tmat = np.zeros((128, EV_G), np.float32)
        tflat = ev_t.reshape(NG, EV_G)
        mflat = ev_m.reshape(NG, EV_G)
        for k in range(14):
            p = _BPOW[k]
            for g in range(NG):
                tmat[16 * g + k, :] = (tflat[g] ** p) * mflat[g]
        return gidx, tmat.astype(ml_dtypes.bfloat16)

    ones16 = np.zeros((128, 8), np.float32)
    for g in range(NG):
        ones16[16 * g:16 * (g + 1), g] = 1.0

    ident_np = np.eye(128, dtype=np.float32)
    in_maps = []
    for k in range(NCORES):
        sl = slice(k * EV_PER_CORE, (k + 1) * EV_PER_CORE)
        gidx, tmat = pack_core(u_all[sl], v_all[sl], tt[sl])
        in_maps.append(
            {
                "zv_all": zv,
                "zv_i": np.ascontiguousarray(zv[k * 256:(k + 1) * 256]),
                "gtab": gtab,
                "gidx": gidx,
                "tmat": tmat,
                "ones16": ones16,
                "tb": tb,
                "t2b": t2b,
                "ident": ident_np,
            }
        )
    return in_maps, (float(t0), float(tn), E)


def _np_event_total(inputs, core):
    """float64 reference event-distance sum for one core's slice."""
    z0 = np.asarray(inputs["z0"], np.float64)
    v0 = np.asarray(inputs["v0"], np.float64)
    uv = np.asarray(inputs["data_uv"], np.int64)
    tt = np.asarray(inputs["data_t"], np.float64)
    sl = slice(core * EV_PER_CORE, (core + 1) * EV_PER_CORE)
    u, v, t = uv[sl, 0], uv[sl, 1], tt[sl]
    dx = (z0[u, 0] - z0[v, 0]) + (v0[u, 0] - v0[v, 0]) * t
    dy = (z0[u, 1] - z0[v, 1]) + (v0[u, 1] - v0[v, 1]) * t
    return np.sqrt(dx * dx + dy * dy).sum()


def _combine(core_outs, beta, t0, tn, E):
    """core_outs: list of [128, 24] float32 partial-sum tensors."""
    exp_sum = 0.0
    ev_sum = 0.0
    for o in core_outs:
        o = np.asarray(o, dtype=np.float64)
        exp_sum += o[:, 0 : ITILES * S].sum()
        ev_sum += o[:, 20].sum()
    b = float(beta)
    dt = (tn - t0) / S
    event_intensity = E * b - ev_sum
    non_event = np.exp(b) * (exp_sum - S * N) / 2.0 * dt
    return np.float32(event_intensity - 1.0 * non_event)


def kernel(**inputs) -> np.ndarray:
    from concourse.bass_utils import run_bass_kernel_spmd

    nc = _build()
    in_maps, (t0, tn, E) = _marshal(inputs)
    res = run_bass_kernel_spmd(nc, in_maps, core_ids=list(range(NCORES)))
    beta = float(np.asarray(inputs["beta"]).reshape(-1)[0])
    out = _combine([r["out_p"] for r in res.results], beta, t0, tn, E)
    return np.asarray(out, dtype=np.float32)


# revision 23
# speedup vs baseline: 1.0618x; 1.0034x over previous
"""Trainium2 Bass kernel for BasicEuclideanDistModel (gnn_message_passing).

Math:
  result = sum_e (beta - ||dz_e + dv_e t_e||)
           - dt * sum_{i<j, s} exp(beta - ||z_i(t_s) - z_j(t_s)||)

Device strategy (8 cores, data parallel):
  * Non-event term: full NxN pairwise distances (halved on host).
    d^2(i,j,s) = F_i(s) . G_j  (K=8 inner product, G time-independent).
    One [8,128]x[8,2048] matmul (fp32r) per (i-tile, s) computes the
    d^2 supertile; DVE relu clamps rounding negatives, ACT computes
    sqrt then exp(-d) with fused row sums.  Each core owns 2 of the
    16 i-tiles, all j, all 10 samples.
  * Event term, split across two independent engines working in
    parallel (events of one u-node always stay together):
    - gpsimd share: d^2(u,v,t) = sum_k A_k(u) B_k(t) C_k(v), a
      14-channel trilinear decomposition with B_k in {1,t,t^2}.
      Events form 8 groups (one per Q7 tile); partition 16g+k holds
      channel k.  ONE ap_gather (SBUF gather, ~27.5ns/idx/core,
      shared index list per group) fetches A_k per segment and C_k
      per event from a [128, N, 2] bf16 channel table; DVE forms
      P = A*C*T (host T = B_k(t)*mask), PE reduces channels with a
      block-ones stationary, one ACT sqrt row-sum -> acc col 20.
    - SWDGE share: baseline scheme -- events grouped by u into
      segments laid out [128, SPD, SLOTD]; dma_gather fetches 256B
      rows (u per segment, v per slot, ~3.8ns/desc aggregate); DVE
      distance algebra, ACT sqrt row-sum -> acc col 21.  Pad slots
      use v=u, t=0 (exactly 0 contribution).
  * beta enters only as a scalar factor / offset -> folded in on host.
  Host combines 8 cores' [128, 24] partial-sum tensors (pure unshard/
  reduction of partials).
"""

import os
import numpy as np


def _import_concourse():
    try:
        import concourse  # noqa: F401
    except ImportError:
        import sys

        for p in ("/opt/trn_rl_repo", "/root/.axon_site/_ro/trn_rl_repo"):
            if os.path.isdir(p) and p not in sys.path:
                sys.path.insert(0, p)


_import_concourse()

from contextlib import ExitStack  # noqa: E402

import concourse.bacc as bacc  # noqa: E402
import concourse.mybir as mybir  # noqa: E402
import concourse.tile as tile  # noqa: E402
from concourse.tile_rust import add_dep_helper  # noqa: E402

N = 2048          # nodes
S = 10            # Riemann samples
NCORES = 8
ITILES = 2        # 128-row i-tiles per core
EV_PER_CORE = 200000 // NCORES       # real events per core

# ---- gpsimd (ap_gather) event share ----
NG = 8            # groups (one per Q7 tile of 16 partitions)
SLOT_G = 6        # event slots per segment
SEG_G = 384       # segments per group
EV_G = SEG_G * SLOT_G                # 2304 event slots per group
NIDX = SEG_G + EV_G                  # 2688 gather indices per group
GP_TARGET = 14000                    # target events on the gpsimd side

# ---- SWDGE (dma_gather) event share ----
SLOT_D = 6        # event slots per segment
SPD = 18          # segments per partition
C_EV = SPD * SLOT_D                  # 108 event columns per partition
NSEG = 128 * SPD                     # 2304 segments per core
SEG_OPS = 2       # seg-gather split into this many dma_gather ops
SEG_PER_OP = NSEG // SEG_OPS
EV_CHUNKS = 6     # v-side gather ops (round-robin over SWDGE queues)
EV_CC = C_EV // EV_CHUNKS            # 18 event columns per chunk
EV_PER_CHUNK = 128 * EV_CC
SEG_CC = SPD // 2                    # segments per half (for algebra views)
GELEM = 64        # gather element size in f32 (256B rows)

F32 = mybir.dt.float32
F32R = mybir.dt.float32r
BF16 = mybir.dt.bfloat16
I16 = mybir.dt.int16
AF = mybir.ActivationFunctionType
OP = mybir.AluOpType

_CACHE: dict = {}
_DBG_SPLIT: list = []


def _tt(nc, out, in0, in1, op):
    return nc.vector.tensor_tensor(out, in0, in1, op=op)


def _build():
    if "nc" in _CACHE:
        return _CACHE["nc"]

    nc = bacc.Bacc(
        "TRN2", target_bir_lowering=False, debug=False, enable_asserts=False,
        num_swdge_queues=4,
    )

    zv_all = nc.dram_tensor("zv_all", [N, 4], F32, kind="ExternalInput").ap()
    zv_pad = nc.dram_tensor("zv_pad", [N, GELEM], F32, kind="ExternalInput").ap()
    zv_i = nc.dram_tensor("zv_i", [ITILES * 128, 4], F32, kind="ExternalInput").ap()
    gtab_d = nc.dram_tensor("gtab", [128, N, 2], BF16, kind="ExternalInput").ap()
    gidx_d = nc.dram_tensor("gidx", [128, NIDX // 16], I16,
                            kind="ExternalInput").ap()
    tmat_d = nc.dram_tensor("tmat", [128, EV_G], BF16, kind="ExternalInput").ap()
    ones16_d = nc.dram_tensor("ones16", [128, 8], F32, kind="ExternalInput").ap()
    ev_u = nc.dram_tensor(
        "ev_u", [128, SEG_OPS, SEG_PER_OP // 16], I16, kind="ExternalInput"
    ).ap()
    ev_v = nc.dram_tensor(
        "ev_v", [128, EV_CHUNKS, EV_PER_CHUNK // 16], I16, kind="ExternalInput"
    ).ap()
    ev_t = nc.dram_tensor("ev_t", [128, C_EV], F32, kind="ExternalInput").ap()
    tb_d = nc.dram_tensor("tb", [128, S], F32, kind="ExternalInput").ap()
    t2b_d = nc.dram_tensor("t2b", [128, S], F32, kind="ExternalInput").ap()
    ident_d = nc.dram_tensor("ident", [128, 128], F32, kind="ExternalInput").ap()
    out_p = nc.dram_tensor("out_p", [128, 24], F32, kind="ExternalOutput").ap()

    with tile.TileContext(nc) as tc, ExitStack() as ctx:
        cpool = ctx.enter_context(tc.tile_pool(name="const", bufs=1))
        evpool = ctx.enter_context(tc.tile_pool(name="ev", bufs=1))

        # ---------------- input loads ----------------
        # gather table + indices first: the single gpsimd ap_gather (and
        # its library load) is the long pole of the event side
        gtab = evpool.tile([128, N, 2], BF16)
        nc.sync.dma_start(gtab[:], gtab_d)
        gidx = evpool.tile([128, NIDX // 16], I16)
        nc.sync.dma_start(gidx[:], gidx_d)
        u_sb = evpool.tile([128, SEG_OPS, SEG_PER_OP // 16], I16)
        for so in range(SEG_OPS):
            nc.sync.dma_start(u_sb[:, so, :], ev_u[:, so, :])
        v_sb = evpool.tile([128, EV_CHUNKS, EV_PER_CHUNK // 16], I16)
        for ch in range(EV_CHUNKS):
            nc.sync.dma_start(v_sb[:, ch, :], ev_v[:, ch, :])
        tmat = evpool.tile([128, EV_G], BF16)
        nc.sync.dma_start(tmat[:], tmat_d)
        t_sb = evpool.tile([128, C_EV], F32)
        nc.sync.dma_start(t_sb[:], ev_t)
        ones16_f = evpool.tile([128, 8], F32)
        nc.sync.dma_start(ones16_f[:], ones16_d)
        zv_sb = cpool.tile([128, 16, 4], F32)        # all nodes, j-side
        nc.sync.dma_start(zv_sb[:], zv_all.rearrange("(c p) d -> p c d", p=128))
        zvi_sb = cpool.tile([128, ITILES, 4], F32)   # this core's i rows
        nc.sync.dma_start(zvi_sb[:], zv_i.rearrange("(c p) d -> p c d", p=128))
        tb = cpool.tile([128, S], F32)
        nc.sync.dma_start(tb[:], tb_d)
        t2b = cpool.tile([128, S], F32)
        nc.sync.dma_start(t2b[:], t2b_d)
        ident = cpool.tile([128, 128], F32)
        nc.sync.dma_start(ident[:], ident_d)

        acc = cpool.tile([128, 24], F32)
        nc.vector.memset(acc[:], 0.0)

        # ---------------- event gathers ----------------
        # gpsimd share: one ap_gather; table row n = [A_k(n), C_k(n)]
        # (d=2 bf16 = 4B per index); segment slots use component 0,
        # event slots component 1
        gout = evpool.tile([128, NIDX, 2], BF16)
        nc.gpsimd.ap_gather(
            gout[:], gtab[:], gidx[:],
            channels=128, num_elems=N, d=2, num_idxs=NIDX,
        )
        ones16 = evpool.tile([128, 8], BF16)
        nc.vector.tensor_copy(ones16[:], ones16_f[:])

        # SWDGE share: u-side one 256B row per SEGMENT; v-side one row
        # per event slot (pads gather v=u, t=0 -> exactly 0)
        d2all = evpool.tile([128, C_EV, 1], F32)
        seg = evpool.tile([128, SPD, GELEM], F32)
        for so in range(SEG_OPS):
            nc.gpsimd.dma_gather(
                seg[:, so * (SPD // SEG_OPS):(so + 1) * (SPD // SEG_OPS), :],
                zv_pad, u_sb[:, so, :], SEG_PER_OP, SEG_PER_OP, GELEM,
                single_packet=False, queue_num=so % 4,
            )
        dvg = ctx.enter_context(tc.tile_pool(name="dvg", bufs=4))
        b_tiles = []
        for ch in range(EV_CHUNKS):
            B = dvg.tile([128, EV_CC, GELEM], F32, tag="B", name="B")
            nc.gpsimd.dma_gather(
                B[:], zv_pad, v_sb[:, ch, :], EV_PER_CHUNK, EV_PER_CHUNK, GELEM,
                single_packet=False, queue_num=ch % 4,
            )
            b_tiles.append(B)

        def emit_dma_event_math(ch, scratch_pool):
            B = b_tiles[ch]
            sc = EV_CC // SLOT_D                     # segments per chunk
            q0 = ch * sc
            shape4 = [128, sc, SLOT_D, 1]
            tse = (
                t_sb[:, ch * EV_CC:(ch + 1) * EV_CC]
                .rearrange("p (q j) -> p q j", j=SLOT_D)
                .unsqueeze(3)
            )

            def sv(d):  # seg channel d view broadcast over the slots
                return (
                    seg[:, q0:q0 + sc, d:d + 1]
                    .unsqueeze(2)
                    .to_broadcast(shape4)
                )

            def bv(d):  # B channel d view
                return B[:, :, d:d + 1].rearrange(
                    "p (q j) d -> p q j d", j=SLOT_D
                )

            dzx = scratch_pool.tile(shape4, F32, tag="w", name="dzx")
            dvx = scratch_pool.tile(shape4, F32, tag="w", name="dvx")
            dzy = scratch_pool.tile(shape4, F32, tag="w", name="dzy")
            dvy = scratch_pool.tile(shape4, F32, tag="w", name="dvy")
            _tt(nc, dzx[:], sv(0), bv(0), OP.subtract)
            _tt(nc, dvx[:], sv(2), bv(2), OP.subtract)
            _tt(nc, dvx[:], dvx[:], tse, OP.mult)
            _tt(nc, dzx[:], dzx[:], dvx[:], OP.add)          # dx
            _tt(nc, dzy[:], sv(1), bv(1), OP.subtract)
            _tt(nc, dvy[:], sv(3), bv(3), OP.subtract)
            _tt(nc, dvy[:], dvy[:], tse, OP.mult)
            _tt(nc, dzy[:], dzy[:], dvy[:], OP.add)          # dy
            _tt(nc, dzx[:], dzx[:], dzx[:], OP.mult)
            _tt(nc, dzy[:], dzy[:], dzy[:], OP.mult)
            d2v = d2all[:, ch * EV_CC:(ch + 1) * EV_CC, :].rearrange(
                "p (q j) d -> p q j d", j=SLOT_D
            )
            _tt(nc, d2v, dzx[:], dzy[:], OP.add)             # d^2

        # ---------------- j features  F[p, chunk, 0:8] ----------------
        # [1, a, b, c, zx, vx, zy, vy]; padded to 32 for the PE transpose
        F = cpool.tile([128, 16, 32], F32)
        zx = zv_sb[:, :, 0:1]
        zy = zv_sb[:, :, 1:2]
        vx = zv_sb[:, :, 2:3]
        vy = zv_sb[:, :, 3:4]
        s1 = cpool.tile([128, 16, 1], F32)
        nc.vector.memset(F[:, :, 0:1], 1.0)
        _tt(nc, F[:, :, 1:2], zx, zx, OP.mult)           # a = zx^2 + zy^2
        _tt(nc, s1[:], zy, zy, OP.mult)
        _tt(nc, F[:, :, 1:2], F[:, :, 1:2], s1[:], OP.add)
        s2 = cpool.tile([128, 16, 1], F32)
        _tt(nc, F[:, :, 2:3], zx, vx, OP.mult)           # b = 2(zx vx + zy vy)
        _tt(nc, s2[:], zy, vy, OP.mult)
        _tt(nc, F[:, :, 2:3], F[:, :, 2:3], s2[:], OP.add)
        nc.vector.tensor_scalar_mul(F[:, :, 2:3], F[:, :, 2:3], 2.0)
        s3 = cpool.tile([128, 16, 1], F32)
        _tt(nc, F[:, :, 3:4], vx, vx, OP.mult)           # c = vx^2 + vy^2
        _tt(nc, s3[:], vy, vy, OP.mult)
        _tt(nc, F[:, :, 3:4], F[:, :, 3:4], s3[:], OP.add)
        nc.vector.tensor_copy(F[:, :, 4:5], zx)
        nc.vector.tensor_copy(F[:, :, 5:6], vx)
        nc.vector.tensor_copy(F[:, :, 6:7], zy)
        nc.vector.tensor_copy(F[:, :, 7:8], vy)

        # ---------------- i features  L[p, it, s, 0:8] ----------------
        # [r, 1, t, t^2, -2x, -2tx, -2y, -2ty]
        L = cpool.tile([128, ITILES, S, 32], F32)
        izx = zvi_sb[:, :, 0:1]
        izy = zvi_sb[:, :, 1:2]
        ivx = zvi_sb[:, :, 2:3]
        ivy = zvi_sb[:, :, 3:4]
        ia = cpool.tile([128, ITILES, 1], F32)
        ib = cpool.tile([128, ITILES, 1], F32)
        ic = cpool.tile([128, ITILES, 1], F32)
        s4 = cpool.tile([128, ITILES, 1], F32)
        _tt(nc, ia[:], izx, izx, OP.mult)
        _tt(nc, s4[:], izy, izy, OP.mult)
        _tt(nc, ia[:], ia[:], s4[:], OP.add)
        s5 = cpool.tile([128, ITILES, 1], F32)
        _tt(nc, ib[:], izx, ivx, OP.mult)
        _tt(nc, s5[:], izy, ivy, OP.mult)
        _tt(nc, ib[:], ib[:], s5[:], OP.add)
        nc.vector.tensor_scalar_mul(ib[:], ib[:], 2.0)
        s6 = cpool.tile([128, ITILES, 1], F32)
        _tt(nc, ic[:], ivx, ivx, OP.mult)
        _tt(nc, s6[:], ivy, ivy, OP.mult)
        _tt(nc, ic[:], ic[:], s6[:], OP.add)

        def b_i(v):  # [128, ITILES, 1] -> [128, ITILES, S, 1]
            return v.unsqueeze(2).to_broadcast([128, ITILES, S, 1])

        tv = tb.unsqueeze(1).unsqueeze(3).to_broadcast([128, ITILES, S, 1])
        t2v = t2b.unsqueeze(1).unsqueeze(3).to_broadcast([128, ITILES, S, 1])

        nc.vector.memset(L[:, :, :, 1:2], 1.0)
        nc.vector.tensor_copy(L[:, :, :, 2:3], tv)
        nc.vector.tensor_copy(L[:, :, :, 3:4], t2v)
        Lx = cpool.tile([128, ITILES, S, 1], F32)
        _tt(nc, Lx[:], b_i(ivx), tv, OP.mult)            # x_i(s) = zx + vx t
        _tt(nc, Lx[:], Lx[:], b_i(izx), OP.add)
        nc.vector.tensor_scalar_mul(L[:, :, :, 4:5], Lx[:], -2.0)
        _tt(nc, L[:, :, :, 5:6], L[:, :, :, 4:5], tv, OP.mult)
        Ly = cpool.tile([128, ITILES, S, 1], F32)
        _tt(nc, Ly[:], b_i(ivy), tv, OP.mult)
        _tt(nc, Ly[:], Ly[:], b_i(izy), OP.add)
        nc.vector.tensor_scalar_mul(L[:, :, :, 6:7], Ly[:], -2.0)
        _tt(nc, L[:, :, :, 7:8], L[:, :, :, 6:7], tv, OP.mult)
        Lr = cpool.tile([128, ITILES, S, 1], F32)
        _tt(nc, L[:, :, :, 0:1], b_i(ib), tv, OP.mult)   # r = a + b t + c t^2
        _tt(nc, L[:, :, :, 0:1], L[:, :, :, 0:1], b_i(ia), OP.add)
        _tt(nc, Lr[:], b_i(ic), t2v, OP.mult)
        _tt(nc, L[:, :, :, 0:1], L[:, :, :, 0:1], Lr[:], OP.add)

        # ---------------- transposes (PE) ----------------
        T2 = cpool.tile([8, N], F32R)                    # G_j rows
        L2 = cpool.tile([8, ITILES * S, 128], F32R)      # F_i(s) rows
        with tc.tile_pool(name="tp", bufs=4, space="PSUM") as tpp:
            for c in range(16):
                pt = tpp.tile([32, 128], F32, tag="pt", name="pt")
                nc.tensor.transpose(pt[:], F[:, c, :], ident[:])
                nc.vector.tensor_copy(T2[:, c * 128:(c + 1) * 128], pt[0:8, :])
            for it in range(ITILES):
                for s in range(S):
                    pt = tpp.tile([32, 128], F32, tag="pt", name="pt")
                    nc.tensor.transpose(pt[:], L[:, it, s, :], ident[:])
                    nc.vector.tensor_copy(L2[:, it * S + s, :], pt[0:8, :])

        # gpsimd-share P = A * C * T views
        P = evpool.tile([128, SEG_G, SLOT_G, 1], BF16)
        shape4g = [128, SEG_G, SLOT_G, 1]
        a_view = gout[:, 0:SEG_G, 0:1].unsqueeze(2).to_broadcast(shape4g)
        c_view = gout[:, SEG_G:NIDX, 1:2].rearrange(
            "p (q j) d -> p q j d", j=SLOT_G
        )
        t_view = tmat.rearrange("p (q j) -> p q j", j=SLOT_G).unsqueeze(3)
        d_ev = evpool.tile([128, C_EV, 1], F32)

        # ---------------- main pairwise loop ----------------
        sq_insts = [[] for _ in range(ITILES)]
        ex_insts = [[] for _ in range(ITILES)]
        with tc.tile_pool(name="qp", bufs=2, space="PSUM") as qpool, \
                tc.tile_pool(name="wp", bufs=12) as wpool:
            for it in range(ITILES):
                for s in range(S):
                    q = qpool.tile([128, N], F32, tag="q", name="q")
                    for kk in range(4):
                        nc.tensor.matmul(
                            q[:, kk * 512:(kk + 1) * 512],
                            L2[:, it * S + s, :],
                            T2[:, kk * 512:(kk + 1) * 512],
                            start=True, stop=True,
                        )
                    w = wpool.tile([128, N], BF16, tag="w", name="w")
                    nc.vector.tensor_scalar_max(w[:], q[:], 0.0)
                    col = it * S + s
                    sq = nc.scalar.activation(w[:], w[:], AF.Sqrt)
                    ex = nc.scalar.activation(
                        w[:], w[:], AF.Exp, scale=-1.0,
                        accum_out=acc[:, col:col + 1],
                    )
                    sq_insts[it].append(sq)
                    ex_insts[it].append(ex)

            # ---- event tails, at the END of every engine stream ----
            # gpsimd share: DVE product, PE channel reduce (single PSUM
            # tile, PSUM->SBUF relu copies between rounds), ACT sqrt
            _tt(nc, P[:], a_view, c_view, OP.mult)
            _tt(nc, P[:], P[:], t_view, OP.mult)
            q_ev = qpool.tile([128, N], F32, tag="q", name="q")
            ev_d2 = evpool.tile([8, EV_G], F32)
            pm = P[:].rearrange("p q j d -> p (q j d)")
            nmm = (EV_G + 511) // 512
            for r in range(nmm):
                c0 = (r % 4) * 512
                cw = min(512, EV_G - 512 * r)
                nc.tensor.matmul(
                    q_ev[0:8, c0:c0 + cw],
                    ones16[:],
                    pm[:, 512 * r:512 * r + cw],
                    start=True, stop=True,
                )
                nc.vector.tensor_scalar_max(
                    ev_d2[:, 512 * r:512 * r + cw],
                    q_ev[0:8, c0:c0 + cw],
                    0.0,
                )
            w_ev = evpool.tile([8, EV_G], BF16)
            ev_sq_g = nc.scalar.activation(
                w_ev[:], ev_d2[:], AF.Sqrt,
                accum_out=acc[0:8, 20:21],
            )

            # SWDGE share: distance algebra per chunk, then one sqrt
            for ch in range(EV_CHUNKS):
                emit_dma_event_math(ch, wpool)
            ev_sq_d = nc.scalar.activation(
                d_ev[:], d2all[:], AF.Sqrt, accum_out=acc[:, 21:22]
            )

            # ACT phase order: sqrt(i0) exp(i0) sqrt(i1) exp(i1) ev_g ev_d.
            # The event sqrts land last: their PE/DVE inputs are only
            # ready near the end of the main loop, and must not gate the
            # exp phases.
            order = (
                sq_insts[0] + ex_insts[0] + sq_insts[1] + ex_insts[1]
                + [ev_sq_g, ev_sq_d]
            )
            for a, b in zip(order[1:], order[:-1]):
                add_dep_helper(a.ins, b.ins, reason="act table phase order")

            nc.sync.dma_start(out_p, acc[:])

    nc.compile()
    _CACHE["nc"] = nc
    return nc


# trilinear channels: (A_k(u), B_k(t) power, C_k(v)); a = zx^2+zy^2,
# b = 2(zx vx + zy vy), c = vx^2+vy^2
# feature columns: [1, a, b, c, zx, zy, vx, vy]
_ACH = [1, 0, 4, 5, 2, 0, 4, 6, 5, 7, 3, 0, 6, 7]   # A feature index
_ASC = [1., 1., -2., -2., 1., 1., -2., -2., -2., -2., 1., 1., -2., -2.]
_BPOW = [0, 0, 0, 0, 1, 1, 1, 1, 1, 1, 2, 2, 2, 2]  # power of t
_CCH = [0, 1, 4, 5, 0, 2, 6, 4, 7, 5, 0, 3, 6, 7]   # C feature index


def _node_features(zv):
    zx, zy, vx, vy = zv[:, 0], zv[:, 1], zv[:, 2], zv[:, 3]
    a = zx * zx + zy * zy
    b = 2.0 * (zx * vx + zy * vy)
    c = vx * vx + vy * vy
    one = np.ones_like(a)
    return np.stack([one, a, b, c, zx, zy, vx, vy], axis=1)  # [N, 8]


def _marshal(inputs):
    import ml_dtypes

    z0 = np.asarray(inputs["z0"], dtype=np.float32)
    v0 = np.asarray(inputs["v0"], dtype=np.float32)
    uv = np.asarray(inputs["data_uv"], dtype=np.int32)
    tt = np.asarray(inputs["data_t"], dtype=np.float32)
    t0 = np.float32(np.asarray(inputs["t0"]).reshape(-1)[0])
    tn = np.float32(np.asarray(inputs["tn"]).reshape(-1)[0])

    zv = np.ascontiguousarray(np.concatenate([z0, v0], axis=1)).astype(np.float32)
    dt = np.float32((tn - t0) / np.float32(S))
    tmid = (t0 + (np.arange(S, dtype=np.float32) + np.float32(0.5)) * dt).astype(
        np.float32
    )
    tb = np.ascontiguousarray(np.broadcast_to(tmid, (128, S))).astype(np.float32)
    t2b = (tb * tb).astype(np.float32)

    zv_pad = np.zeros((N, GELEM), np.float32)
    zv_pad[:, 0:4] = zv

    feats = _node_features(zv.astype(np.float64)).astype(np.float32)  # [N, 8]
    gtab = np.zeros((128, N, 2), np.float32)
    for k in range(14):
        for g in range(NG):
            gtab[16 * g + k, :, 0] = _ASC[k] * feats[:, _ACH[k]]
            gtab[16 * g + k, :, 1] = feats[:, _CCH[k]]
    gtab = gtab.astype(ml_dtypes.bfloat16)

    E = uv.shape[0]
    assert E <= NCORES * EV_PER_CORE
    u_all = uv[:, 0].astype(np.int64)
    v_all = uv[:, 1].astype(np.int64)

    def split_core(u, v, t):
        """Assign each u-node's events wholly to the gpsimd or the SWDGE
        share; fill gpsimd groups (balanced) up to GP_TARGET events."""
        counts = np.bincount(u, minlength=N)
        order = np.argsort(-counts, kind="stable")
        g_ev = np.zeros(NG, np.int64)
        g_seg = np.zeros(NG, np.int64)
        node_g = np.full(N, -1, np.int64)   # -1 -> SWDGE share
        total = 0
        for n in order:
            c = int(counts[n])
            if c == 0 or total >= GP_TARGET:
                continue
            segs = -(-c // SLOT_G)
            g = int(np.argmin(g_ev))
            if g_ev[g] + segs * SLOT_G > EV_G or g_seg[g] + segs > SEG_G:
                continue
            node_g[n] = g
            g_ev[g] += segs * SLOT_G        # reserve whole segments
            g_seg[g] += segs
            total += c
        return node_g

    def pack_gp(u, v, t, node_g):
        """gpsimd share: wrapped gather index list + T tensor."""
        sel = node_g[u] >= 0
        us, vs, ts = u[sel], v[sel], t[sel]
        gs = node_g[us]
        order = np.argsort(us, kind="stable")
        us, vs, ts, gs = us[order], vs[order], ts[order], gs[order]

        seg_u = np.zeros((NG, SEG_G), np.int64)
        ev_vv = np.zeros((NG, SEG_G, SLOT_G), np.int64)
        ev_tt = np.zeros((NG, SEG_G, SLOT_G), np.float32)
        ev_mm = np.zeros((NG, SEG_G, SLOT_G), np.float32)
        seg_cnt = np.zeros(NG, np.int64)
        i = 0
        while i < len(us):
            j = i
            while j < len(us) and us[j] == us[i]:
                j += 1
            g = int(gs[i])
            for s0 in range(i, j, SLOT_G):
                q = seg_cnt[g]
                assert q < SEG_G, "gp segment overflow"
                seg_cnt[g] += 1
                e0 = min(s0 + SLOT_G, j)
                seg_u[g, q] = us[i]
                ev_vv[g, q, : e0 - s0] = vs[s0:e0]
                ev_tt[g, q, : e0 - s0] = ts[s0:e0]
                ev_mm[g, q, : e0 - s0] = 1.0
            i = j
        idx_flat = np.concatenate(
            [seg_u, ev_vv.reshape(NG, EV_G)], axis=1
        ).astype(np.int16)
        gidx = np.zeros((128, NIDX // 16), np.int16)
        for g in range(NG):
            gidx[16 * g:16 * (g + 1), :] = (
                idx_flat[g].reshape(NIDX // 16, 16).T
            )
        tmat = np.zeros((128, EV_G), np.float32)
        tflat = ev_tt.reshape(NG, EV_G)
        mflat = ev_mm.reshape(NG, EV_G)
        for k in range(14):
            p = _BPOW[k]
            for g in range(NG):
                tmat[16 * g + k, :] = (tflat[g] ** p) * mflat[g]
        return gidx, tmat.astype(ml_dtypes.bfloat16)

    def pack_dma(u, v, t, node_g):
        """SWDGE share: baseline segment layout (pads v=u, t=0)."""
        sel = node_g[u] < 0
        us, vs, ts = u[sel], v[sel], t[sel]
        order = np.argsort(us, kind="stable")
        us, vs, ts = us[order], vs[order], ts[order]
        starts = np.flatnonzero(np.r_[True, us[1:] != us[:-1]])
        ends = np.r_[starts[1:], len(us)]
        seg_nodes = np.zeros((128, SPD), np.int16)
        v_slots = np.zeros((128, SPD, SLOT_D), np.int16)
        t_slots = np.zeros((128, SPD, SLOT_D), np.float32)
        counts = np.zeros(128, np.int64)
        i = 0
        for s0, e0 in zip(starts, ends):
            n = us[s0]
            for j in range(s0, e0, SLOT_D):
                p = i % 128
                q = counts[p]
                counts[p] += 1
                assert q < SPD, "dma segment overflow; raise SPD"
                i += 1
                seg_nodes[p, q] = n
                va = vs[j:min(j + SLOT_D, e0)]
                ta = ts[j:min(j + SLOT_D, e0)]
                v_slots[p, q, :] = n
                v_slots[p, q, : len(va)] = va
                t_slots[p, q, : len(ta)] = ta
        return (
            seg_nodes,
            v_slots.reshape(128, C_EV),
            t_slots.reshape(128, C_EV),
        )

    def wrap16(x, nops, per_op):
        w = x.reshape(nops, per_op // 16, 16).transpose(2, 0, 1)
        return np.ascontiguousarray(np.tile(w, (8, 1, 1)))

    ones16 = np.zeros((128, 8), np.float32)
    for g in range(NG):
        ones16[16 * g:16 * (g + 1), g] = 1.0

    ident_np = np.eye(128, dtype=np.float32)
    in_maps = []
    _DBG_SPLIT.clear()
    for k in range(NCORES):
        sl = slice(k * EV_PER_CORE, (k + 1) * EV_PER_CORE)
        u, v, t = u_all[sl], v_all[sl], tt[sl]
        node_g = split_core(u, v, t)
        zv64 = zv.astype(np.float64)

        def _dsum(mask):
            uu, vv, tt_ = u[mask], v[mask], t[mask]
            dx = (zv64[uu, 0] - zv64[vv, 0]) + (zv64[uu, 2] - zv64[vv, 2]) * tt_
            dy = (zv64[uu, 1] - zv64[vv, 1]) + (zv64[uu, 3] - zv64[vv, 3]) * tt_
            return float(np.sqrt(dx * dx + dy * dy).sum())

        _DBG_SPLIT.append(
            (_dsum(node_g[u] >= 0), _dsum(node_g[u] < 0), int((node_g[u] >= 0).sum()))
        )
        gidx, tmat = pack_gp(u, v, t, node_g)
        seg_nodes, v_slots, t_slots = pack_dma(u, v, t, node_g)
        seg_list = seg_nodes.T.reshape(-1)
        v_list = (
            v_slots.reshape(128, EV_CHUNKS, EV_CC)
            .transpose(1, 2, 0)
            .reshape(-1)
        )
        in_maps.append(
            {
                "zv_all": zv,
                "zv_pad": zv_pad,
                "zv_i": np.ascontiguousarray(zv[k * 256:(k + 1) * 256]),
                "gtab": gtab,
                "gidx": gidx,
                "tmat": tmat,
                "ones16": ones16,
                "ev_u": wrap16(seg_list, SEG_OPS, SEG_PER_OP),
                "ev_v": wrap16(v_list, EV_CHUNKS, EV_PER_CHUNK),
                "ev_t": np.ascontiguousarray(t_slots),
                "tb": tb,
                "t2b": t2b,
                "ident": ident_np,
            }
        )
    return in_maps, (float(t0), float(tn), E)


def _np_event_total(inputs, core):
    """float64 reference event-distance sum for one core's slice."""
    z0 = np.asarray(inputs["z0"], np.float64)
    v0 = np.asarray(inputs["v0"], np.float64)
    uv = np.asarray(inputs["data_uv"], np.int64)
    tt = np.asarray(inputs["data_t"], np.float64)
    sl = slice(core * EV_PER_CORE, (core + 1) * EV_PER_CORE)
    u, v, t = uv[sl, 0], uv[sl, 1], tt[sl]
    dx = (z0[u, 0] - z0[v, 0]) + (v0[u, 0] - v0[v, 0]) * t
    dy = (z0[u, 1] - z0[v, 1]) + (v0[u, 1] - v0[v, 1]) * t
    return np.sqrt(dx * dx + dy * dy).sum()


def _combine(core_outs, beta, t0, tn, E):
    """core_outs: list of [128, 24] float32 partial-sum tensors."""
    exp_sum = 0.0
    ev_sum = 0.0
    for o in core_outs:
        o = np.asarray(o, dtype=np.float64)
        exp_sum += o[:, 0 : ITILES * S].sum()
        ev_sum += o[:, 20].sum() + o[:, 21].sum()
    b = float(beta)
    dt = (tn - t0) / S
    event_intensity = E * b - ev_sum
    non_event = np.exp(b) * (exp_sum - S * N) / 2.0 * dt
    return np.float32(event_intensity - 1.0 * non_event)


def kernel(**inputs) -> np.ndarray:
    from concourse.bass_utils import run_bass_kernel_spmd

    nc = _build()
    in_maps, (t0, tn, E) = _marshal(inputs)
    res = run_bass_kernel_spmd(nc, in_maps, core_ids=list(range(NCORES)))
    beta = float(np.asarray(inputs["beta"]).reshape(-1)[0])
    out = _combine([r["out_p"] for r in res.results], beta, t0, tn, E)
    return np.asarray(out, dtype=np.float32)


# revision 25
# speedup vs baseline: 1.6453x; 1.5495x over previous
"""Trainium2 Bass kernel for BasicEuclideanDistModel (gnn_message_passing).

Math:
  result = sum_e (beta - ||dz_e + dv_e t_e||)
           - dt * sum_{i<j, s} exp(beta - ||z_i(t_s) - z_j(t_s)||)

Device strategy (8 cores, data parallel):
  * Non-event term: full NxN pairwise distances (halved on host).
    d^2(i,j,s) = F_i(s) . G_j  (K=8 inner product, G time-independent).
    One [8,128]x[8,2048] matmul (fp32r) per (i-tile, s) computes the
    d^2 supertile; DVE relu clamps rounding negatives, ACT computes
    sqrt then exp(-d) with fused row sums.  Each core owns 2 of the
    16 i-tiles, all j, all 10 samples.
  * Event term, split across two independent engines working in
    parallel (events of one u-node always stay together):
    - gpsimd share: d^2(u,v,t) = sum_k A_k(u) B_k(t) C_k(v), a
      14-channel trilinear decomposition with B_k in {1,t,t^2}.
      Events form 8 groups (one per Q7 tile); partition 16g+k holds
      channel k.  ONE ap_gather (SBUF gather, ~27.5ns/idx/core,
      shared index list per group) fetches A_k per segment and C_k
      per event from a [128, N, 2] bf16 channel table; DVE forms
      P = A*C*T (host T = B_k(t)*mask), PE reduces channels with a
      block-ones stationary, one ACT sqrt row-sum -> acc col 20.
    - SWDGE share: baseline scheme -- events grouped by u into
      segments laid out [128, SPD, SLOTD]; dma_gather fetches 256B
      rows (u per segment, v per slot, ~3.8ns/desc aggregate); DVE
      distance algebra, ACT sqrt row-sum -> acc col 21.  Pad slots
      use v=u, t=0 (exactly 0 contribution).
  * beta enters only as a scalar factor / offset -> folded in on host.
  Host combines 8 cores' [128, 24] partial-sum tensors (pure unshard/
  reduction of partials).
"""

import os
import numpy as np


def _import_concourse():
    try:
        import concourse  # noqa: F401
    except ImportError:
        import sys

        for p in ("/opt/trn_rl_repo", "/root/.axon_site/_ro/trn_rl_repo"):
            if os.path.isdir(p) and p not in sys.path:
                sys.path.insert(0, p)


_import_concourse()

from contextlib import ExitStack  # noqa: E402

import concourse.bacc as bacc  # noqa: E402
import concourse.mybir as mybir  # noqa: E402
import concourse.tile as tile  # noqa: E402
from concourse.tile_rust import add_dep_helper  # noqa: E402

N = 2048          # nodes
S = 10            # Riemann samples
NCORES = 8
ITILES = 2        # 128-row i-tiles per core
EV_PER_CORE = 200000 // NCORES       # real events per core

# ---- gpsimd (ap_gather) event share: the highest-count u-nodes ----
NG = 8            # groups (one per Q7 tile of 16 partitions)
SLOT_G = 6        # event slots per segment
SEG_G = 208       # segments per group
EV_G = SEG_G * SLOT_G                # 1248 event slots per group
NIDX = SEG_G + EV_G                  # 1456 gather indices per group
GP_TARGET = 8000                     # target events on the gpsimd side

# ---- SWDGE (dma_gather) event share ----
SLOT_D = 6        # event slots per segment
SPD = 28          # segments per partition
C_EV = SPD * SLOT_D                  # 168 event columns per partition
NSEG = 128 * SPD                     # 3584 segments per core
SEG_OPS = 2       # seg-gather split into this many dma_gather ops
SEG_PER_OP = NSEG // SEG_OPS
EV_CHUNKS = 4     # v-side gather ops (one per SWDGE queue)
EV_CC = C_EV // EV_CHUNKS            # 42 event columns per chunk
EV_PER_CHUNK = 128 * EV_CC
GELEM = 64        # gather element size in f32 (256B rows)

F32 = mybir.dt.float32
F32R = mybir.dt.float32r
BF16 = mybir.dt.bfloat16
I16 = mybir.dt.int16
AF = mybir.ActivationFunctionType
OP = mybir.AluOpType

_CACHE: dict = {}
_DBG_SPLIT: list = []


def _tt(nc, out, in0, in1, op):
    return nc.vector.tensor_tensor(out, in0, in1, op=op)


def _build():
    if "nc" in _CACHE:
        return _CACHE["nc"]

    nc = bacc.Bacc(
        "TRN2", target_bir_lowering=False, debug=False, enable_asserts=False,
        num_swdge_queues=4,
    )

    zv_all = nc.dram_tensor("zv_all", [N, 4], F32, kind="ExternalInput").ap()
    zv_pad = nc.dram_tensor("zv_pad", [N, GELEM], F32, kind="ExternalInput").ap()
    zv_i = nc.dram_tensor("zv_i", [ITILES * 128, 4], F32, kind="ExternalInput").ap()
    gtab_d = nc.dram_tensor("gtab", [128, N, 2], BF16, kind="ExternalInput").ap()
    gidx_d = nc.dram_tensor("gidx", [128, NIDX // 16], I16,
                            kind="ExternalInput").ap()
    tmat_d = nc.dram_tensor("tmat", [128, EV_G], BF16, kind="ExternalInput").ap()
    ones16_d = nc.dram_tensor("ones16", [128, 8], F32, kind="ExternalInput").ap()
    ev_u = nc.dram_tensor(
        "ev_u", [128, SEG_OPS, SEG_PER_OP // 16], I16, kind="ExternalInput"
    ).ap()
    ev_v = nc.dram_tensor(
        "ev_v", [128, EV_CHUNKS, EV_PER_CHUNK // 16], I16, kind="ExternalInput"
    ).ap()
    ev_t = nc.dram_tensor("ev_t", [128, C_EV], F32, kind="ExternalInput").ap()
    tb_d = nc.dram_tensor("tb", [128, S], F32, kind="ExternalInput").ap()
    t2b_d = nc.dram_tensor("t2b", [128, S], F32, kind="ExternalInput").ap()
    ident_d = nc.dram_tensor("ident", [128, 128], F32, kind="ExternalInput").ap()
    out_p = nc.dram_tensor("out_p", [128, 24], F32, kind="ExternalOutput").ap()

    with tile.TileContext(nc) as tc, ExitStack() as ctx:
        cpool = ctx.enter_context(tc.tile_pool(name="const", bufs=1))
        evpool = ctx.enter_context(tc.tile_pool(name="ev", bufs=1))

        # ---------------- input loads ----------------
        # gather table + indices first: the single gpsimd ap_gather (and
        # its library load) is the long pole of the event side
        gtab = evpool.tile([128, N, 2], BF16)
        nc.sync.dma_start(gtab[:], gtab_d)
        gidx = evpool.tile([128, NIDX // 16], I16)
        nc.sync.dma_start(gidx[:], gidx_d)
        u_sb = evpool.tile([128, SEG_OPS, SEG_PER_OP // 16], I16)
        for so in range(SEG_OPS):
            nc.sync.dma_start(u_sb[:, so, :], ev_u[:, so, :])
        v_sb = evpool.tile([128, EV_CHUNKS, EV_PER_CHUNK // 16], I16)
        for ch in range(EV_CHUNKS):
            nc.sync.dma_start(v_sb[:, ch, :], ev_v[:, ch, :])
        tmat = evpool.tile([128, EV_G], BF16)
        nc.sync.dma_start(tmat[:], tmat_d)
        t_sb = evpool.tile([128, C_EV], F32)
        nc.sync.dma_start(t_sb[:], ev_t)
        ones16_f = evpool.tile([128, 8], F32)
        nc.sync.dma_start(ones16_f[:], ones16_d)
        zv_sb = cpool.tile([128, 16, 4], F32)        # all nodes, j-side
        nc.sync.dma_start(zv_sb[:], zv_all.rearrange("(c p) d -> p c d", p=128))
        zvi_sb = cpool.tile([128, ITILES, 4], F32)   # this core's i rows
        nc.sync.dma_start(zvi_sb[:], zv_i.rearrange("(c p) d -> p c d", p=128))
        tb = cpool.tile([128, S], F32)
        nc.sync.dma_start(tb[:], tb_d)
        t2b = cpool.tile([128, S], F32)
        nc.sync.dma_start(t2b[:], t2b_d)
        ident = cpool.tile([128, 128], F32)
        nc.sync.dma_start(ident[:], ident_d)

        acc = cpool.tile([128, 24], F32)
        nc.vector.memset(acc[:], 0.0)

        # ---------------- event gathers ----------------
        # gpsimd stream order matters: the SWDGE descriptor GENERATION
        # runs first (its DMA drain proceeds on the DMA engines while the
        # gpsimd engine moves on), then one library reload, then the
        # ap_gather for the gpsimd share.
        # SWDGE share: u-side one 256B row per SEGMENT; v-side one row
        # per event slot (pads gather v=u, t=0 -> exactly 0)
        d2all = evpool.tile([128, C_EV, 1], F32)
        seg = evpool.tile([128, SPD, GELEM], F32)
        for so in range(SEG_OPS):
            nc.gpsimd.dma_gather(
                seg[:, so * (SPD // SEG_OPS):(so + 1) * (SPD // SEG_OPS), :],
                zv_pad, u_sb[:, so, :], SEG_PER_OP, SEG_PER_OP, GELEM,
                single_packet=False, queue_num=so % 4,
            )
        dvg = ctx.enter_context(tc.tile_pool(name="dvg", bufs=4))
        b_tiles = []
        for ch in range(EV_CHUNKS):
            B = dvg.tile([128, EV_CC, GELEM], F32, tag="B", name="B")
            nc.gpsimd.dma_gather(
                B[:], zv_pad, v_sb[:, ch, :], EV_PER_CHUNK, EV_PER_CHUNK, GELEM,
                single_packet=False, queue_num=ch % 4,
            )
            b_tiles.append(B)

        # gpsimd share: one ap_gather; table row n = [A_k(n), C_k(n)]
        # (d=2 bf16 = 4B per index); segment slots use component 0,
        # event slots component 1
        gout = evpool.tile([128, NIDX, 2], BF16)
        nc.gpsimd.ap_gather(
            gout[:], gtab[:], gidx[:],
            channels=128, num_elems=N, d=2, num_idxs=NIDX,
        )
        ones16 = evpool.tile([128, 8], BF16)
        nc.vector.tensor_copy(ones16[:], ones16_f[:])

        def emit_dma_event_math(ch, scratch_pool):
            B = b_tiles[ch]
            sc = EV_CC // SLOT_D                     # segments per chunk
            q0 = ch * sc
            shape4 = [128, sc, SLOT_D, 1]
            tse = (
                t_sb[:, ch * EV_CC:(ch + 1) * EV_CC]
                .rearrange("p (q j) -> p q j", j=SLOT_D)
                .unsqueeze(3)
            )

            def sv(d):  # seg channel d view broadcast over the slots
                return (
                    seg[:, q0:q0 + sc, d:d + 1]
                    .unsqueeze(2)
                    .to_broadcast(shape4)
                )

            def bv(d):  # B channel d view
                return B[:, :, d:d + 1].rearrange(
                    "p (q j) d -> p q j d", j=SLOT_D
                )

            dzx = scratch_pool.tile(shape4, F32, tag="w", name="dzx")
            dvx = scratch_pool.tile(shape4, F32, tag="w", name="dvx")
            dzy = scratch_pool.tile(shape4, F32, tag="w", name="dzy")
            dvy = scratch_pool.tile(shape4, F32, tag="w", name="dvy")
            _tt(nc, dzx[:], sv(0), bv(0), OP.subtract)
            _tt(nc, dvx[:], sv(2), bv(2), OP.subtract)
            _tt(nc, dvx[:], dvx[:], tse, OP.mult)
            _tt(nc, dzx[:], dzx[:], dvx[:], OP.add)          # dx
            _tt(nc, dzy[:], sv(1), bv(1), OP.subtract)
            _tt(nc, dvy[:], sv(3), bv(3), OP.subtract)
            _tt(nc, dvy[:], dvy[:], tse, OP.mult)
            _tt(nc, dzy[:], dzy[:], dvy[:], OP.add)          # dy
            _tt(nc, dzx[:], dzx[:], dzx[:], OP.mult)
            _tt(nc, dzy[:], dzy[:], dzy[:], OP.mult)
            d2v = d2all[:, ch * EV_CC:(ch + 1) * EV_CC, :].rearrange(
                "p (q j) d -> p q j d", j=SLOT_D
            )
            _tt(nc, d2v, dzx[:], dzy[:], OP.add)             # d^2

        # ---------------- j features  F[p, chunk, 0:8] ----------------
        # [1, a, b, c, zx, vx, zy, vy]; padded to 32 for the PE transpose
        F = cpool.tile([128, 16, 32], F32)
        zx = zv_sb[:, :, 0:1]
        zy = zv_sb[:, :, 1:2]
        vx = zv_sb[:, :, 2:3]
        vy = zv_sb[:, :, 3:4]
        s1 = cpool.tile([128, 16, 1], F32)
        nc.vector.memset(F[:, :, 0:1], 1.0)
        _tt(nc, F[:, :, 1:2], zx, zx, OP.mult)           # a = zx^2 + zy^2
        _tt(nc, s1[:], zy, zy, OP.mult)
        _tt(nc, F[:, :, 1:2], F[:, :, 1:2], s1[:], OP.add)
        s2 = cpool.tile([128, 16, 1], F32)
        _tt(nc, F[:, :, 2:3], zx, vx, OP.mult)           # b = 2(zx vx + zy vy)
        _tt(nc, s2[:], zy, vy, OP.mult)
        _tt(nc, F[:, :, 2:3], F[:, :, 2:3], s2[:], OP.add)
        nc.vector.tensor_scalar_mul(F[:, :, 2:3], F[:, :, 2:3], 2.0)
        s3 = cpool.tile([128, 16, 1], F32)
        _tt(nc, F[:, :, 3:4], vx, vx, OP.mult)           # c = vx^2 + vy^2
        _tt(nc, s3[:], vy, vy, OP.mult)
        _tt(nc, F[:, :, 3:4], F[:, :, 3:4], s3[:], OP.add)
        nc.vector.tensor_copy(F[:, :, 4:5], zx)
        nc.vector.tensor_copy(F[:, :, 5:6], vx)
        nc.vector.tensor_copy(F[:, :, 6:7], zy)
        nc.vector.tensor_copy(F[:, :, 7:8], vy)

        # ---------------- i features  L[p, it, s, 0:8] ----------------
        # [r, 1, t, t^2, -2x, -2tx, -2y, -2ty]
        L = cpool.tile([128, ITILES, S, 32], F32)
        izx = zvi_sb[:, :, 0:1]
        izy = zvi_sb[:, :, 1:2]
        ivx = zvi_sb[:, :, 2:3]
        ivy = zvi_sb[:, :, 3:4]
        ia = cpool.tile([128, ITILES, 1], F32)
        ib = cpool.tile([128, ITILES, 1], F32)
        ic = cpool.tile([128, ITILES, 1], F32)
        s4 = cpool.tile([128, ITILES, 1], F32)
        _tt(nc, ia[:], izx, izx, OP.mult)
        _tt(nc, s4[:], izy, izy, OP.mult)
        _tt(nc, ia[:], ia[:], s4[:], OP.add)
        s5 = cpool.tile([128, ITILES, 1], F32)
        _tt(nc, ib[:], izx, ivx, OP.mult)
        _tt(nc, s5[:], izy, ivy, OP.mult)
        _tt(nc, ib[:], ib[:], s5[:], OP.add)
        nc.vector.tensor_scalar_mul(ib[:], ib[:], 2.0)
        s6 = cpool.tile([128, ITILES, 1], F32)
        _tt(nc, ic[:], ivx, ivx, OP.mult)
        _tt(nc, s6[:], ivy, ivy, OP.mult)
        _tt(nc, ic[:], ic[:], s6[:], OP.add)

        def b_i(v):  # [128, ITILES, 1] -> [128, ITILES, S, 1]
            return v.unsqueeze(2).to_broadcast([128, ITILES, S, 1])

        tv = tb.unsqueeze(1).unsqueeze(3).to_broadcast([128, ITILES, S, 1])
        t2v = t2b.unsqueeze(1).unsqueeze(3).to_broadcast([128, ITILES, S, 1])

        nc.vector.memset(L[:, :, :, 1:2], 1.0)
        nc.vector.tensor_copy(L[:, :, :, 2:3], tv)
        nc.vector.tensor_copy(L[:, :, :, 3:4], t2v)
        Lx = cpool.tile([128, ITILES, S, 1], F32)
        _tt(nc, Lx[:], b_i(ivx), tv, OP.mult)            # x_i(s) = zx + vx t
        _tt(nc, Lx[:], Lx[:], b_i(izx), OP.add)
        nc.vector.tensor_scalar_mul(L[:, :, :, 4:5], Lx[:], -2.0)
        _tt(nc, L[:, :, :, 5:6], L[:, :, :, 4:5], tv, OP.mult)
        Ly = cpool.tile([128, ITILES, S, 1], F32)
        _tt(nc, Ly[:], b_i(ivy), tv, OP.mult)
        _tt(nc, Ly[:], Ly[:], b_i(izy), OP.add)
        nc.vector.tensor_scalar_mul(L[:, :, :, 6:7], Ly[:], -2.0)
        _tt(nc, L[:, :, :, 7:8], L[:, :, :, 6:7], tv, OP.mult)
        Lr = cpool.tile([128, ITILES, S, 1], F32)
        _tt(nc, L[:, :, :, 0:1], b_i(ib), tv, OP.mult)   # r = a + b t + c t^2
        _tt(nc, L[:, :, :, 0:1], L[:, :, :, 0:1], b_i(ia), OP.add)
        _tt(nc, Lr[:], b_i(ic), t2v, OP.mult)
        _tt(nc, L[:, :, :, 0:1], L[:, :, :, 0:1], Lr[:], OP.add)

        # ---------------- transposes (PE) ----------------
        T2 = cpool.tile([8, N], F32R)                    # G_j rows
        L2 = cpool.tile([8, ITILES * S, 128], F32R)      # F_i(s) rows
        with tc.tile_pool(name="tp", bufs=4, space="PSUM") as tpp:
            for c in range(16):
                pt = tpp.tile([32, 128], F32, tag="pt", name="pt")
                nc.tensor.transpose(pt[:], F[:, c, :], ident[:])
                nc.vector.tensor_copy(T2[:, c * 128:(c + 1) * 128], pt[0:8, :])
            for it in range(ITILES):
                for s in range(S):
                    pt = tpp.tile([32, 128], F32, tag="pt", name="pt")
                    nc.tensor.transpose(pt[:], L[:, it, s, :], ident[:])
                    nc.vector.tensor_copy(L2[:, it * S + s, :], pt[0:8, :])

        # gpsimd-share P = A * C * T views
        P = evpool.tile([128, SEG_G, SLOT_G, 1], BF16)
        shape4g = [128, SEG_G, SLOT_G, 1]
        a_view = gout[:, 0:SEG_G, 0:1].unsqueeze(2).to_broadcast(shape4g)
        c_view = gout[:, SEG_G:NIDX, 1:2].rearrange(
            "p (q j) d -> p q j d", j=SLOT_G
        )
        t_view = tmat.rearrange("p (q j) -> p q j", j=SLOT_G).unsqueeze(3)
        d_ev = evpool.tile([128, C_EV, 1], F32)

        # ---------------- main pairwise loop ----------------
        sq_insts = [[] for _ in range(ITILES)]
        ex_insts = [[] for _ in range(ITILES)]
        with tc.tile_pool(name="qp", bufs=2, space="PSUM") as qpool, \
                tc.tile_pool(name="wp", bufs=12) as wpool:
            for it in range(ITILES):
                for s in range(S):
                    q = qpool.tile([128, N], F32, tag="q", name="q")
                    for kk in range(4):
                        nc.tensor.matmul(
                            q[:, kk * 512:(kk + 1) * 512],
                            L2[:, it * S + s, :],
                            T2[:, kk * 512:(kk + 1) * 512],
                            start=True, stop=True,
                        )
                    w = wpool.tile([128, N], BF16, tag="w", name="w")
                    nc.vector.tensor_scalar_max(w[:], q[:], 0.0)
                    col = it * S + s
                    sq = nc.scalar.activation(w[:], w[:], AF.Sqrt)
                    ex = nc.scalar.activation(
                        w[:], w[:], AF.Exp, scale=-1.0,
                        accum_out=acc[:, col:col + 1],
                    )
                    sq_insts[it].append(sq)
                    ex_insts[it].append(ex)

            # ---- event tails, at the END of every engine stream ----
            # gpsimd share: DVE product, PE channel reduce (single PSUM
            # tile, PSUM->SBUF relu copies between rounds), ACT sqrt
            _tt(nc, P[:], a_view, c_view, OP.mult)
            _tt(nc, P[:], P[:], t_view, OP.mult)
            q_ev = qpool.tile([128, N], F32, tag="q", name="q")
            ev_d2 = evpool.tile([8, EV_G], F32)
            pm = P[:].rearrange("p q j d -> p (q j d)")
            nmm = (EV_G + 511) // 512
            for r in range(nmm):
                c0 = (r % 4) * 512
                cw = min(512, EV_G - 512 * r)
                nc.tensor.matmul(
                    q_ev[0:8, c0:c0 + cw],
                    ones16[:],
                    pm[:, 512 * r:512 * r + cw],
                    start=True, stop=True,
                )
                nc.vector.tensor_scalar_max(
                    ev_d2[:, 512 * r:512 * r + cw],
                    q_ev[0:8, c0:c0 + cw],
                    0.0,
                )
            w_ev = evpool.tile([8, EV_G], BF16)
            ev_sq_g = nc.scalar.activation(
                w_ev[:], ev_d2[:], AF.Sqrt,
                accum_out=acc[0:8, 20:21],
            )

            # SWDGE share: distance algebra per chunk, then one sqrt
            for ch in range(EV_CHUNKS):
                emit_dma_event_math(ch, wpool)
            ev_sq_d = nc.scalar.activation(
                d_ev[:], d2all[:], AF.Sqrt, accum_out=acc[:, 21:22]
            )

            # ACT phase order: sqrt(i0) exp(i0) sqrt(i1) exp(i1) ev_g ev_d.
            # The event sqrts land last: their PE/DVE inputs are only
            # ready near the end of the main loop, and must not gate the
            # exp phases.
            order = (
                sq_insts[0] + ex_insts[0] + sq_insts[1] + ex_insts[1]
                + [ev_sq_g, ev_sq_d]
            )
            for a, b in zip(order[1:], order[:-1]):
                add_dep_helper(a.ins, b.ins, reason="act table phase order")

            nc.sync.dma_start(out_p, acc[:])

    nc.compile()
    _CACHE["nc"] = nc
    return nc


# trilinear channels: (A_k(u), B_k(t) power, C_k(v)); a = zx^2+zy^2,
# b = 2(zx vx + zy vy), c = vx^2+vy^2
# feature columns: [1, a, b, c, zx, zy, vx, vy]
_ACH = [1, 0, 4, 5, 2, 0, 4, 6, 5, 7, 3, 0, 6, 7]   # A feature index
_ASC = [1., 1., -2., -2., 1., 1., -2., -2., -2., -2., 1., 1., -2., -2.]
_BPOW = [0, 0, 0, 0, 1, 1, 1, 1, 1, 1, 2, 2, 2, 2]  # power of t
_CCH = [0, 1, 4, 5, 0, 2, 6, 4, 7, 5, 0, 3, 6, 7]   # C feature index


def _node_features(zv):
    zx, zy, vx, vy = zv[:, 0], zv[:, 1], zv[:, 2], zv[:, 3]
    a = zx * zx + zy * zy
    b = 2.0 * (zx * vx + zy * vy)
    c = vx * vx + vy * vy
    one = np.ones_like(a)
    return np.stack([one, a, b, c, zx, zy, vx, vy], axis=1)  # [N, 8]


def _marshal(inputs):
    import ml_dtypes

    z0 = np.asarray(inputs["z0"], dtype=np.float32)
    v0 = np.asarray(inputs["v0"], dtype=np.float32)
    uv = np.asarray(inputs["data_uv"], dtype=np.int32)
    tt = np.asarray(inputs["data_t"], dtype=np.float32)
    t0 = np.float32(np.asarray(inputs["t0"]).reshape(-1)[0])
    tn = np.float32(np.asarray(inputs["tn"]).reshape(-1)[0])

    zv = np.ascontiguousarray(np.concatenate([z0, v0], axis=1)).astype(np.float32)
    dt = np.float32((tn - t0) / np.float32(S))
    tmid = (t0 + (np.arange(S, dtype=np.float32) + np.float32(0.5)) * dt).astype(
        np.float32
    )
    tb = np.ascontiguousarray(np.broadcast_to(tmid, (128, S))).astype(np.float32)
    t2b = (tb * tb).astype(np.float32)

    zv_pad = np.zeros((N, GELEM), np.float32)
    zv_pad[:, 0:4] = zv

    feats = _node_features(zv.astype(np.float64)).astype(np.float32)  # [N, 8]
    gtab = np.zeros((128, N, 2), np.float32)
    for k in range(14):
        for g in range(NG):
            gtab[16 * g + k, :, 0] = _ASC[k] * feats[:, _ACH[k]]
            gtab[16 * g + k, :, 1] = feats[:, _CCH[k]]
    gtab = gtab.astype(ml_dtypes.bfloat16)

    E = uv.shape[0]
    assert E <= NCORES * EV_PER_CORE
    u_all = uv[:, 0].astype(np.int64)
    v_all = uv[:, 1].astype(np.int64)

    def split_core(u, v, t):
        """Assign each u-node's events wholly to the gpsimd or the SWDGE
        share; fill gpsimd groups (balanced) up to GP_TARGET events."""
        counts = np.bincount(u, minlength=N)
        order = np.argsort(-counts, kind="stable")
        g_ev = np.zeros(NG, np.int64)
        g_seg = np.zeros(NG, np.int64)
        node_g = np.full(N, -1, np.int64)   # -1 -> SWDGE share
        total = 0
        for n in order:
            c = int(counts[n])
            if c == 0 or total >= GP_TARGET:
                continue
            segs = -(-c // SLOT_G)
            g = int(np.argmin(g_ev))
            if g_ev[g] + segs * SLOT_G > EV_G or g_seg[g] + segs > SEG_G:
                continue
            node_g[n] = g
            g_ev[g] += segs * SLOT_G        # reserve whole segments
            g_seg[g] += segs
            total += c
        return node_g

    def pack_gp(u, v, t, node_g):
        """gpsimd share: wrapped gather index list + T tensor."""
        sel = node_g[u] >= 0
        us, vs, ts = u[sel], v[sel], t[sel]
        gs = node_g[us]
        order = np.argsort(us, kind="stable")
        us, vs, ts, gs = us[order], vs[order], ts[order], gs[order]

        seg_u = np.zeros((NG, SEG_G), np.int64)
        ev_vv = np.zeros((NG, SEG_G, SLOT_G), np.int64)
        ev_tt = np.zeros((NG, SEG_G, SLOT_G), np.float32)
        ev_mm = np.zeros((NG, SEG_G, SLOT_G), np.float32)
        seg_cnt = np.zeros(NG, np.int64)
        i = 0
        while i < len(us):
            j = i
            while j < len(us) and us[j] == us[i]:
                j += 1
            g = int(gs[i])
            for s0 in range(i, j, SLOT_G):
                q = seg_cnt[g]
                assert q < SEG_G, "gp segment overflow"
                seg_cnt[g] += 1
                e0 = min(s0 + SLOT_G, j)
                seg_u[g, q] = us[i]
                ev_vv[g, q, : e0 - s0] = vs[s0:e0]
                ev_tt[g, q, : e0 - s0] = ts[s0:e0]
                ev_mm[g, q, : e0 - s0] = 1.0
            i = j
        idx_flat = np.concatenate(
            [seg_u, ev_vv.reshape(NG, EV_G)], axis=1
        ).astype(np.int16)
        gidx = np.zeros((128, NIDX // 16), np.int16)
        for g in range(NG):
            gidx[16 * g:16 * (g + 1), :] = (
                idx_flat[g].reshape(NIDX // 16, 16).T
            )
        tmat = np.zeros((128, EV_G), np.float32)
        tflat = ev_tt.reshape(NG, EV_G)
        mflat = ev_mm.reshape(NG, EV_G)
        for k in range(14):
            p = _BPOW[k]
            for g in range(NG):
                tmat[16 * g + k, :] = (tflat[g] ** p) * mflat[g]
        return gidx, tmat.astype(ml_dtypes.bfloat16)

    def pack_dma(u, v, t, node_g):
        """SWDGE share: baseline segment layout (pads v=u, t=0)."""
        sel = node_g[u] < 0
        us, vs, ts = u[sel], v[sel], t[sel]
        order = np.argsort(us, kind="stable")
        us, vs, ts = us[order], vs[order], ts[order]
        starts = np.flatnonzero(np.r_[True, us[1:] != us[:-1]])
        ends = np.r_[starts[1:], len(us)]
        seg_nodes = np.zeros((128, SPD), np.int16)
        v_slots = np.zeros((128, SPD, SLOT_D), np.int16)
        t_slots = np.zeros((128, SPD, SLOT_D), np.float32)
        counts = np.zeros(128, np.int64)
        i = 0
        for s0, e0 in zip(starts, ends):
            n = us[s0]
            for j in range(s0, e0, SLOT_D):
                p = i % 128
                q = counts[p]
                counts[p] += 1
                assert q < SPD, "dma segment overflow; raise SPD"
                i += 1
                seg_nodes[p, q] = n
                va = vs[j:min(j + SLOT_D, e0)]
                ta = ts[j:min(j + SLOT_D, e0)]
                v_slots[p, q, :] = n
                v_slots[p, q, : len(va)] = va
                t_slots[p, q, : len(ta)] = ta
        return (
            seg_nodes,
            v_slots.reshape(128, C_EV),
            t_slots.reshape(128, C_EV),
        )

    def wrap16(x, nops, per_op):
        w = x.reshape(nops, per_op // 16, 16).transpose(2, 0, 1)
        return np.ascontiguousarray(np.tile(w, (8, 1, 1)))

    ones16 = np.zeros((128, 8), np.float32)
    for g in range(NG):
        ones16[16 * g:16 * (g + 1), g] = 1.0

    ident_np = np.eye(128, dtype=np.float32)
    in_maps = []
    _DBG_SPLIT.clear()
    for k in range(NCORES):
        sl = slice(k * EV_PER_CORE, (k + 1) * EV_PER_CORE)
        u, v, t = u_all[sl], v_all[sl], tt[sl]
        node_g = split_core(u, v, t)
        zv64 = zv.astype(np.float64)

        def _dsum(mask):
            uu, vv, tt_ = u[mask], v[mask], t[mask]
            dx = (zv64[uu, 0] - zv64[vv, 0]) + (zv64[uu, 2] - zv64[vv, 2]) * tt_
            dy = (zv64[uu, 1] - zv64[vv, 1]) + (zv64[uu, 3] - zv64[vv, 3]) * tt_
            return float(np.sqrt(dx * dx + dy * dy).sum())

        _DBG_SPLIT.append(
            (_dsum(node_g[u] >= 0), _dsum(node_g[u] < 0), int((node_g[u] >= 0).sum()))
        )
        gidx, tmat = pack_gp(u, v, t, node_g)
        seg_nodes, v_slots, t_slots = pack_dma(u, v, t, node_g)
        seg_list = seg_nodes.T.reshape(-1)
        v_list = (
            v_slots.reshape(128, EV_CHUNKS, EV_CC)
            .transpose(1, 2, 0)
            .reshape(-1)
        )
        in_maps.append(
            {
                "zv_all": zv,
                "zv_pad": zv_pad,
                "zv_i": np.ascontiguousarray(zv[k * 256:(k + 1) * 256]),
                "gtab": gtab,
                "gidx": gidx,
                "tmat": tmat,
                "ones16": ones16,
                "ev_u": wrap16(seg_list, SEG_OPS, SEG_PER_OP),
                "ev_v": wrap16(v_list, EV_CHUNKS, EV_PER_CHUNK),
                "ev_t": np.ascontiguousarray(t_slots),
                "tb": tb,
                "t2b": t2b,
                "ident": ident_np,
            }
        )
    return in_maps, (float(t0), float(tn), E)


def _np_event_total(inputs, core):
    """float64 reference event-distance sum for one core's slice."""
    z0 = np.asarray(inputs["z0"], np.float64)
    v0 = np.asarray(inputs["v0"], np.float64)
    uv = np.asarray(inputs["data_uv"], np.int64)
    tt = np.asarray(inputs["data_t"], np.float64)
    sl = slice(core * EV_PER_CORE, (core + 1) * EV_PER_CORE)
    u, v, t = uv[sl, 0], uv[sl, 1], tt[sl]
    dx = (z0[u, 0] - z0[v, 0]) + (v0[u, 0] - v0[v, 0]) * t
    dy = (z0[u, 1] - z0[v, 1]) + (v0[u, 1] - v0[v, 1]) * t
    return np.sqrt(dx * dx + dy * dy).sum()


def _combine(core_outs, beta, t0, tn, E):
    """core_outs: list of [128, 24] float32 partial-sum tensors."""
    exp_sum = 0.0
    ev_sum = 0.0
    for o in core_outs:
        o = np.asarray(o, dtype=np.float64)
        exp_sum += o[:, 0 : ITILES * S].sum()
        ev_sum += o[:, 20].sum() + o[:, 21].sum()
    b = float(beta)
    dt = (tn - t0) / S
    event_intensity = E * b - ev_sum
    non_event = np.exp(b) * (exp_sum - S * N) / 2.0 * dt
    return np.float32(event_intensity - 1.0 * non_event)


def kernel(**inputs) -> np.ndarray:
    from concourse.bass_utils import run_bass_kernel_spmd

    nc = _build()
    in_maps, (t0, tn, E) = _marshal(inputs)
    res = run_bass_kernel_spmd(nc, in_maps, core_ids=list(range(NCORES)))
    beta = float(np.asarray(inputs["beta"]).reshape(-1)[0])
    out = _combine([r["out_p"] for r in res.results], beta, t0, tn, E)
    return np.asarray(out, dtype=np.float32)
